# revision 42
# baseline (speedup 1.0000x reference)
"""Trainium2 Bass kernel for the butterfly-CNN problem (nn_CNNLayer_30296699306356).

Network (see problem reference): input conv (k=2,s=2, 1->8 ch) + 10 butterfly
conv levels (k=2,s=2, channels double each level, relu, zero biases) + a
per-block dense matmul (1024 blocks of [8,2]) at the end.

Strategy (memory-regime; weights are ~358 MB fp32 dominated by levels 8-10):
  - Levels 5..9 run in bf16, level 10 in fp8 e3m4 (scale 4, folded into
    fea_dense); activations bf16 from level 5 on, fp32 PSUM accumulation.
    Measured rel err vs the fp32 reference: ~1.4e-2 (threshold 2e-2).
  - Levels in..8 are replicated on all 8 cores; levels 9/10 shard the OUTPUT
    channels across the cores (1/8 of the dominant weight traffic per core).
  - The x9 shard exchange (each core's 512 channels -> all cores) is done
    with direct peer-to-peer SWDGE remote DMAs (XOR slot layout: slot d on
    core r holds the shard of core r^d), NOT an NRT collective: the NRT
    AllGather costs ~40 us of latency; the raw remote DMA exchange costs a
    few us. A compile-time-inserted prelude AllGather (bir_kernel_barrier)
    makes the cross-core SBUF writes safe. Per-slot semaphores let level 10
    consume each peer shard as it lands (slot-pipelined contraction).
  - Level 10's weight chunks are pre-permuted per core into slot order, so
    the contraction walks slots 0..7 (own shard first, no wait).
  - DMA queue split: the big weight stream (wmid/w9/w10) runs on the sync
    HWDGE queue with enough pool buffers that it never stalls; w8 (whose
    pool WAR-waits on L8 compute) runs on the scalar HWDGE queue so it
    cannot head-of-line-block the stream.
  - Final block einsum on the vector engine (broadcast mult + grouped
    reduce); output layout [B, o, block], transposed on host.

kernel(**inputs) takes the FULL unsharded inputs and returns the FULL output.
"""

import ml_dtypes
import numpy as np

NCORES = 8
B = 16
P = 128
C = 8
NLVL = 10
BF16 = ml_dtypes.bfloat16
E3M4 = ml_dtypes.float8_e3m4
W10_SCALE = 4.0  # f10 quantized as e3m4(4*w); 1/4 folded into fea_dense
GATHER = "remote"  # "remote" (peer DMA exchange) or "cc" (NRT AllGather)
# slot d on core r holds the x9 shard of core PERM[r][d] = r ^ PSI[d]; PSI
# measured empirically with a rank-broadcast probe kernel (the driver's
# logical->physical nc remap makes the relative-XOR addressing land slots
# 4..7 with bit 1 flipped: cross-die first hops swap the RMTV lane pair)
PSI = [0, 1, 2, 3, 6, 7, 4, 5]
PERM = [[r ^ PSI[d] for d in range(NCORES)] for r in range(NCORES)]

_CACHE = {}


# ---------------------------------------------------------------- host prep

def _host_prep(inputs):
    """Build the per-core input maps (numpy only)."""
    ind = np.ascontiguousarray(np.asarray(inputs["in_data"], np.float32))
    f = {l: np.asarray(inputs[f"f{l}"], np.float32) for l in range(1, NLVL + 1)}
    f0 = np.asarray(inputs["in_filter"], np.float32)     # [2, 1, 8]
    fd = np.asarray(inputs["fea_dense"], np.float32)     # [1024, 8, 2]

    shared = {}
    # r0 [32, 64, 16]: r0[row, wHi, b] = in[b, wHi*32 + row]
    shared["r0"] = np.ascontiguousarray(
        ind[:, :, 0].reshape(B, 64, 32).transpose(2, 1, 0))

    # w0 [32, 128]: rows (2*wsub + k), cols (wsub*8 + co)
    w0 = np.zeros((32, 128), np.float32)
    for wsub in range(16):
        for k in range(2):
            w0[2 * wsub + k, wsub * 8:wsub * 8 + 8] = f0[k, 0, :]
    shared["w0"] = w0

    # packed levels 1..4 stacked: wpk [4, 128, 128]
    wpk = np.zeros((4, 128, 128), np.float32)
    for lvl in range(1, 5):
        cin = 2 ** (lvl - 1) * C
        cout = 2 ** lvl * C
        s_out = (128 // cin) // 2
        for wso in range(s_out):
            for k in range(2):
                wsi = 2 * wso + k
                wpk[lvl - 1, wsi * cin:(wsi + 1) * cin,
                    wso * cout:(wso + 1) * cout] = f[lvl][k]
    shared["wpk"] = wpk

    # w5/w6/w7 mega-packed [128, 10752] bf16 (kt-major per level), one DMA
    w5v = f[5].astype(BF16).reshape(2, 1, 128, 256)
    w6v = f[6].astype(BF16).reshape(2, 2, 128, 512)
    w7v = f[7].astype(BF16).reshape(2, 4, 128, 1024)
    shared["wmid"] = np.ascontiguousarray(np.concatenate([
        w5v.transpose(2, 0, 1, 3).reshape(128, 512),
        w6v.transpose(2, 0, 1, 3).reshape(128, 2048),
        w7v.transpose(2, 0, 1, 3).reshape(128, 8192)], axis=1))

    # f8/f9/f10 output-channel shards, packed into 4-ci-tile chunks:
    # [nchunks, 128, 4, cout_shard]; chunk m = k*(cin//512) + q, cit = q*4+j
    def shard_pack(fl, cin, dt=BF16):
        sh = fl.shape[-1] // NCORES
        out = []
        flb = fl.astype(dt)
        for r in range(NCORES):
            blk = flb[:, :, r * sh:(r + 1) * sh]
            v = blk.reshape(2, cin // 512, 4, 128, sh).transpose(0, 1, 3, 2, 4)
            out.append(np.ascontiguousarray(
                v.reshape(2 * (cin // 512), 128, 4, sh)))
        return out

    # f8 is REPLICATED (cheaper than a second exchange): co-major chunks
    # [4, 128, kt=16, co=512], kt = k*8 + cit
    f8b = f[8].astype(BF16)
    w8full = np.stack([
        np.ascontiguousarray(
            f8b[:, :, c * 512:(c + 1) * 512]
            .reshape(2, 8, 128, 512).transpose(2, 0, 1, 3).reshape(128, 16, 512))
        for c in range(4)])
    shared["w8"] = w8full

    w9s = shard_pack(f[9], 2048)                      # 8 chunks of [128, 4, 512]
    w10s = shard_pack(f[10] * W10_SCALE, 4096, E3M4)  # 16 chunks of [128, 4, 1024]
    if GATHER == "remote":
        # reorder per core into slot order: new[2d+k] = old[k*8 + PERM[r][d]]
        w10s = [np.ascontiguousarray(
                    w[[k * 8 + PERM[r][d] for d in range(8) for k in range(2)]])
                for r, w in enumerate(w10s)]

    # fea_dense shard, per-o flattened, tiled over the 16 batch partitions;
    # carries the 1/W10_SCALE compensation for the e3m4 level-10 weights
    fds = []
    for r in range(NCORES):
        blk = fd[r * 128:(r + 1) * 128] / W10_SCALE        # [128, 8, 2]
        flat = blk.transpose(2, 0, 1).reshape(2, 1024)     # [o, 1024]
        fds.append(np.ascontiguousarray(
            np.broadcast_to(flat[None], (B, 2, 1024))))

    in_maps = []
    for r in range(NCORES):
        m = dict(shared)
        m["w9"] = w9s[r]
        m["w10"] = w10s[r]
        m["fdt"] = fds[r]
        in_maps.append(m)
    return in_maps


# ---------------------------------------------------------------- bass build

def _build():
    import concourse.bass as bass
    import concourse.mybir as mybir
    import concourse.tile as tile
    from concourse import bacc, library_config

    f32 = mybir.dt.float32
    bf16 = mybir.dt.bfloat16
    fp8 = mybir.dt.float8e3
    RELU = mybir.ActivationFunctionType.Relu

    nc = bacc.Bacc("TRN2", target_bir_lowering=False, debug=False,
                   num_devices=NCORES)

    def inp(name, shape, dt=f32):
        return nc.dram_tensor(name, shape, dt, kind="ExternalInput").ap()

    r0 = inp("r0", [32, 64, 16])
    w0 = inp("w0", [32, 128])
    wpk = inp("wpk", [4, 128, 128])
    wmid = inp("wmid", [128, 10752], bf16)
    w8 = inp("w8", [4, 128, 16, 512], bf16)
    w9 = inp("w9", [8, 128, 4, 512], bf16)
    w10 = inp("w10", [16, 128, 4, 1024], fp8)
    fdt = inp("fdt", [B, 2, 1024])
    out = nc.dram_tensor("out", [B, 2, 128], f32, kind="ExternalOutput").ap()

    remote = GATHER == "remote"

    with tile.TileContext(nc) as tc:
        with (
            tc.tile_pool(name="const", bufs=1) as constp,
            tc.tile_pool(name="actp", bufs=3) as actp,
            tc.tile_pool(name="bigp", bufs=1) as bigp,
            tc.tile_pool(name="w7p", bufs=1) as w7p,
            tc.tile_pool(name="w8p", bufs=2) as w8p,
            tc.tile_pool(name="w9p", bufs=8) as w9p,
            tc.tile_pool(name="w10p", bufs=14) as w10p,
            tc.tile_pool(name="psA", bufs=2, space="PSUM") as psA,
            tc.tile_pool(name="psB", bufs=4, space="PSUM") as psB,
            tc.tile_pool(name="psC", bufs=2, space="PSUM") as psC,
            tc.tile_pool(name="dramp", bufs=1, space="DRAM") as dramp,
        ):
            if remote:
                rsems = [nc.alloc_semaphore(f"xch{d}") for d in range(1, NCORES)]
                lsem = nc.alloc_semaphore("xch_local")
                x9rdy = nc.alloc_semaphore("x9rdy")
                nc.gpsimd.load_library(library_config.remote_dma)
                slot_mms = {d: [] for d in range(1, NCORES)}

            if remote:
                # exchange buffers allocated up front so the SWDGE descriptor
                # generation (slow: ~1.7us per broadcast prep on Q7) can run
                # during the initial weight streaming; the data dependency on
                # x9loc defers to the trigger
                x9loc = bigp.tile([128, 4, 2, 16], bf16, name="x9loc")
                x9sb = bigp.tile([128, 32, 2, 16], bf16, name="x9sb")
                preps = []
                for d in range(1, NCORES):
                    rdests = [None] * NCORES
                    rdests[d] = (0, d)
                    preps.append(nc.gpsimd.remote_dma_broadcast(
                        x9sb[:, 4 * d:4 * d + 4], x9loc[:],
                        remote_sem=rsems[d - 1], local_sem=lsem,
                        rdests=rdests))

            # ---- resident loads (sync queue = the main weight stream)
            r0sb = constp.tile([32, 64, 16], f32, name="r0sb")
            nc.sync.dma_start(r0sb[:], r0)
            w0sb = constp.tile([32, 128], f32, name="w0sb")
            nc.sync.dma_start(w0sb[:], w0)
            wpksb = constp.tile([128, 4, 128], f32, name="wpksb")
            nc.sync.dma_start(wpksb[:], wpk.rearrange("l p c -> p l c"))
            wmidsb = w7p.tile([128, 10752], bf16, name="wmidsb")
            nc.sync.dma_start(wmidsb[:], wmid)
            w5sb = wmidsb[:, 0:512].rearrange("p (t c) -> p t c", c=256)
            w6sb = wmidsb[:, 512:2560].rearrange("p (t c) -> p t c", c=512)
            w7sb = wmidsb[:, 2560:10752].rearrange("p (t c) -> p t c", c=1024)
            fdsb = constp.tile([B, 2, 1024], f32, name="fdsb")
            nc.scalar.dma_start(fdsb[:], fdt)

            # ---- input conv + packed levels 1..4 (all [128, 64, 16])
            xprev = None
            for lvl in range(5):
                # x4 feeds the bf16 level-5 matmul, so cast at the relu
                xn = actp.tile([128, 64, 16], bf16 if lvl == 4 else f32,
                               name=f"x{lvl}", tag="xl")
                for ch in range(2):
                    ps = psA.tile([128, 32, 16], f32, name="psA", tag="psA")
                    if lvl == 0:
                        nc.tensor.matmul(
                            ps[:], w0sb[:], r0sb[:, ch * 32:(ch + 1) * 32, :],
                            start=True, stop=True)
                    else:
                        nc.tensor.matmul(
                            ps[:], wpksb[:, lvl - 1, :],
                            xprev[:, ch * 32:(ch + 1) * 32, :],
                            start=True, stop=True)
                    nc.scalar.activation(
                        xn[:, ch * 32:(ch + 1) * 32, :], ps[:], RELU)
                xprev = xn

            # ---- standard levels (orientation A, weights stationary)
            def std_level(xin, wsb, cin_t, cout_t, w_out, name, out_tile=None):
                # xin [128, cin_t, 2*w_out, 16]; wsb [128, 2*cin_t, co] with
                # kt = k*cin_t + cit; returns [128, cout_t, w_out, 16]
                if out_tile is None:
                    xn = actp.tile([128, cout_t, w_out, 16], bf16,
                                   name=name, tag="xl")
                else:
                    xn = out_tile
                for ct in range(cout_t):
                    ps = psA.tile([128, w_out, 16], f32, name="psA", tag="psA")
                    for cit in range(cin_t):
                        rhs2 = xin[:, cit].rearrange(
                            "p (w two) b -> p two w b", two=2)
                        for k in range(2):
                            nc.tensor.matmul(
                                ps[:],
                                wsb[:, k * cin_t + cit,
                                    ct * 128:(ct + 1) * 128],
                                rhs2[:, k],
                                start=(cit == 0 and k == 0),
                                stop=(cit == cin_t - 1 and k == 1))
                    nc.scalar.activation(xn[:, ct], ps[:], RELU)
                return xn

            x5 = std_level(xprev[:, None], w5sb, 1, 2, 32, "x5")
            x6 = std_level(x5, w6sb, 2, 4, 16, "x6")
            x7 = std_level(x6, w7sb, 4, 8, 8, "x7")

            # ---- level 8 REPLICATED (full 2048 cout), co-major weight stream
            # w8 chunks stream on the scalar queue: their pool WAR-waits on L8
            # compute and must not block the sync queue's w9/w10 stream.
            x8sb = bigp.tile([128, 16, 4, 16], bf16, name="x8sb")
            for c in range(4):
                w8c = w8p.tile([128, 16, 512], bf16, name="w8c", tag="w8c")
                nc.scalar.dma_start(w8c[:], w8[c])
                for ctl in range(4):
                    ps = psA.tile([128, 4, 16], f32, name="psA", tag="psA")
                    for cit in range(8):
                        rhs2 = x7[:, cit].rearrange(
                            "p (w two) b -> p two w b", two=2)
                        for k in range(2):
                            nc.tensor.matmul(
                                ps[:],
                                w8c[:, k * 8 + cit, ctl * 128:(ctl + 1) * 128],
                                rhs2[:, k],
                                start=(cit == 0 and k == 0),
                                stop=(cit == 7 and k == 1))
                    nc.scalar.activation(x8sb[:, c * 4 + ctl], ps[:], RELU)

            # ---- level 9 (512-ch shard, streamed weights, 4 accumulators)
            ps9 = [psB.tile([128, 2, 16], f32, name=f"ps9_{ct}", tag="psB")
                   for ct in range(4)]
            for m in range(8):
                w9c = w9p.tile([128, 4, 512], bf16, name="w9c", tag="w9c")
                nc.sync.dma_start(w9c[:], w9[m])
                k, q = divmod(m, 4)
                for j in range(4):
                    cit = q * 4 + j
                    rhs = x8sb[:, cit].rearrange(
                        "p (w two) b -> p two w b", two=2)[:, k]
                    for ct in range(4):
                        nc.tensor.matmul(
                            ps9[ct][:],
                            w9c[:, j, ct * 128:(ct + 1) * 128],
                            rhs,
                            start=(m == 0 and j == 0),
                            stop=(m == 7 and j == 3))
            if not remote:
                x9loc = bigp.tile([128, 4, 2, 16], bf16, name="x9loc")
            for ct in range(4):
                nc.scalar.activation(x9loc[:, ct], ps9[ct][:], RELU)
            if remote:
                # The early-emitted SWDGE preps had no x9loc producer to
                # defer a RAW edge from, so trigger1 would fire before x9loc
                # exists. A reader spanning all four relu outputs bumps an
                # explicit readiness semaphore for it instead (the relus'
                # own sync-update slots are taken).
                x9scr = bigp.tile([128, 4, 16], bf16, name="x9scr")
                x9cp = nc.vector.tensor_copy(x9scr[:], x9loc[:, :, 1, :])

            # ---- x9 shard exchange: push my shard into slot d of peer
            # me^d; peer's slot d gets +2 on rsems[d-1] when the bytes have
            # landed. A second redundant send round (same bytes, same sems)
            # covers the race where a late-starting peer's start-of-kernel
            # sem clear wipes the first round's increments.
            if remote:
                trig1 = nc.gpsimd.trigger_dma(count=None)
                for d in range(1, NCORES):
                    rdests = [None] * NCORES
                    rdests[d] = (0, d)
                    nc.gpsimd.remote_dma_broadcast(
                        x9sb[:, 4 * d:4 * d + 4], x9loc[:],
                        remote_sem=rsems[d - 1], local_sem=lsem,
                        rdests=rdests)
                nc.gpsimd.trigger_dma(count=None)
            else:
                x9sb = bigp.tile([128, 32, 2, 16], bf16, name="x9sb")
                ag9_in = dramp.tile([1, 128, 4, 2, 16], bf16, name="ag9_in")
                ag9_out = dramp.tile([NCORES, 128, 4, 2, 16], bf16,
                                     name="ag9_out", addr_space="Shared")
                nc.scalar.dma_start(ag9_in[0], x9loc[:])
                nc.gpsimd.collective_compute(
                    "AllGather", mybir.AluOpType.bypass,
                    replica_groups=[list(range(NCORES))],
                    ins=[ag9_in.opt()], outs=[ag9_out.opt()])
                for r in range(NCORES):
                    nc.scalar.dma_start(x9sb[:, 4 * r:4 * r + 4], ag9_out[r])

            # ---- level 10 (1024-ch shard, orientation B: acts stationary,
            #      e3m4 weights moving, slot-pipelined over peer shards)
            ps10 = [psC.tile([B, 512], f32, name=f"ps10_{cb}", tag="psC")
                    for cb in range(2)]
            for d in range(NCORES):
                if remote and d == 0:
                    src = x9loc
                    base = 0
                else:
                    src = x9sb
                    base = 4 * d
                for k in range(2):
                    m = 2 * d + k
                    w10c = w10p.tile([128, 4, 1024], fp8, name="w10c",
                                     tag="w10c")
                    nc.sync.dma_start(w10c[:], w10[m])
                    for j in range(4):
                        lhsT = src[:, base + j, k, :]
                        for cb in range(2):
                            mm = nc.tensor.matmul(
                                ps10[cb][:], lhsT,
                                w10c[:, j, cb * 512:(cb + 1) * 512],
                                start=(d == 0 and k == 0 and j == 0),
                                stop=(d == NCORES - 1 and k == 1 and j == 3))
                            if remote and d > 0 and k == 0 and j == 0:
                                # first matmul of each psum chain that touches
                                # slot d; gets the arrival wait attached
                                # post-scheduling (invisible to the tile sim)
                                slot_mms[d].append(mm)
            x10 = bigp.tile([B, 1024], f32, name="x10")
            for cb in range(2):
                nc.scalar.activation(
                    x10[:, cb * 512:(cb + 1) * 512], ps10[cb][:], RELU)

            # ---- final per-block einsum on the vector engine
            # osb laid out [B, o, k]; host transposes to [B, k, o]
            prod = bigp.tile([B, 2, 1024], f32, name="prod")
            osb = bigp.tile([B, 2, 128], f32, name="osb")
            x10b = x10.rearrange("p (one f) -> p one f", one=1).broadcast_to(
                [B, 2, 1024])
            nc.vector.tensor_tensor(
                prod[:], x10b, fdsb[:], mybir.AluOpType.mult)
            nc.vector.tensor_reduce(
                osb[:],
                prod.rearrange("p o (k c) -> p o k c", c=8),
                mybir.AxisListType.X, mybir.AluOpType.add)
            nc.scalar.dma_start(out, osb[:])

    if remote:
        # Cross-core waits are attached AFTER the tile block: the tile
        # scheduling pass simulates a single core and would report the
        # remote-satisfied semaphores as a deadlock.
        #  - a PE-stream wait right before the first matmul touching slot d
        #    gates the whole slot on the peer shard having landed (+2 on
        #    rsems[d-1]); inserted directly into the lowered instruction
        #    list (the matmuls' own wait slots are already taken by the
        #    tile-assigned weight-chunk arrival waits)
        def find_block(ins):
            for blk in nc.main_func.blocks:
                for i, x in enumerate(blk.instructions):
                    if x.name == ins.name:
                        return blk, i
            raise KeyError(ins.name)

        for d in range(1, NCORES):
            w = nc.tensor.wait_ge(rsems[d - 1], 2).ins
            wblk, widx = find_block(w)
            del wblk.instructions[widx]
            # insert before whichever of the slot's chain-first matmuls the
            # scheduler placed earliest
            places = [find_block(mm.ins) for mm in slot_mms[d]]
            mblk, midx = min(places, key=lambda bi: bi[1])
            assert all(b is mblk for b, _ in places)
            mblk.instructions.insert(midx, w)
        #  - trigger1 must not fire descriptors before x9loc is computed.
        #    The x9scr copy reads all four relu outputs; a nop-with-update
        #    placed right after it on the in-order vector stream bumps x9rdy
        #    (no instruction has a free sync-update slot for a second
        #    update, so the increment needs its own instruction)
        ni = nc.vector.nop(nofuse=True).then_inc(x9rdy, 1)
        niblk, niidx = find_block(ni.ins)
        del niblk.instructions[niidx]
        cblk, cidx = find_block(x9cp.ins)
        cblk.instructions.insert(cidx + 1, ni.ins)
        wt = nc.gpsimd.wait_ge(x9rdy, 1).ins
        wblk, widx = find_block(wt)
        del wblk.instructions[widx]
        tblk, tidx = find_block(trig1.ins)
        tblk.instructions.insert(tidx, wt)
        # Register the kernel-entry barrier replica group so compile()
        # inserts its prelude AllGather. NOTHING in the program waits on it:
        # it exists purely so the NEFF contains a collective, which makes
        # the runtime rendezvous the 8 cores' execution starts (without it,
        # profiler arming staggers the starts by milliseconds and every
        # early core idle-waits that long for peer shards).
        nc._bir_kernel_barrier_sem_replica_groups.append(set(range(NCORES)))
        # End-of-kernel drains: hold the epilogue semaphore zeroing until
        # every in-flight increment has landed -- both rounds of every peer's
        # push to me (sem_d reaches 4) and my own sends' local ticks. This
        # makes the between-executions state clean for any launch skew: once
        # a semaphore reads its final value nothing else is in flight, so
        # the epilogue clear cannot lose increments.
        for d in range(1, NCORES):
            nc.gpsimd.wait_ge(rsems[d - 1], 4)
        nc.gpsimd.wait_ge(lsem, 2 * 16 * (NCORES - 1))

    nc.compile()
    return nc


def _build_cleaner():
    """A trivial one-shot NEFF that zeroes the kernel semaphore range.

    The main kernel's cross-core exchange assumes its semaphores start at 0.
    Executions N>=1 get that from execution N-1's epilogue clear (held back
    by the end-of-kernel drains until all increments landed), but the very
    first execution after NEFF load would see whatever junk previous NEFFs
    left in the semaphore file. Running this cleaner once before the first
    real execution makes execution 0 sound too.
    """
    import concourse.mybir as mybir
    import concourse.tile as tile
    from concourse import bacc

    f32 = mybir.dt.float32
    nc = bacc.Bacc("TRN2", target_bir_lowering=False, debug=False,
                   num_devices=NCORES)
    out = nc.dram_tensor("out", [1, 128], f32, kind="ExternalOutput")
    with tile.TileContext(nc) as tc:
        with tc.tile_pool(name="p", bufs=1) as pool:
            z = pool.tile([1, 128], f32, name="z")
            nc.vector.memset(z[:], 0.0)
            nc.sync.dma_start(out.ap(), z[:])
    c = nc.gpsimd.sem_clear(range(100, 255)).ins
    for blk in nc.main_func.blocks:
        for i, x in enumerate(blk.instructions):
            if x.name == c.name:
                del blk.instructions[i]
                break
    tgt = next(b for b in nc.main_func.blocks
               if "tile_context" in b.name and "end" not in b.name)
    tgt.instructions.insert(0, c)
    nc.compile()
    return nc


# ------------------------------------------------------------------- kernel

def kernel(**inputs):
    from concourse.bass_utils import run_bass_kernel_spmd

    in_maps = _host_prep(inputs)
    if "nc" not in _CACHE:
        _CACHE["nc"] = _build()
        if GATHER == "remote":
            # zero the semaphore file (previous NEFFs leave junk), then run
            # one discarded warmup execution: the very first execution after
            # NEFF load delivers remote SBUF writes unreliably (sem
            # increments arrive, some data lanes don't), so its output is
            # garbage; every later execution is sound.
            cl = _build_cleaner()
            run_bass_kernel_spmd(cl, [{} for _ in range(NCORES)],
                                 core_ids=list(range(NCORES)))
            run_bass_kernel_spmd(_CACHE["nc"], in_maps,
                                 core_ids=list(range(NCORES)))
    nc = _CACHE["nc"]
    res = run_bass_kernel_spmd(nc, in_maps, core_ids=list(range(NCORES)))
    parts = [res.results[r]["out"].transpose(0, 2, 1)       # [16, 128, 2]
             for r in range(NCORES)]
    full = np.concatenate(parts, axis=1)                    # [16, 1024, 2]
    return np.ascontiguousarray(full.reshape(B, 2048, 1).astype(np.float32))


# revision 44
# speedup vs baseline: 1.0578x; 1.0578x over previous
"""Trainium2 Bass kernel for the butterfly-CNN problem (nn_CNNLayer_30296699306356).

Network (see problem reference): input conv (k=2,s=2, 1->8 ch) + 10 butterfly
conv levels (k=2,s=2, channels double each level, relu, zero biases) + a
per-block dense matmul (1024 blocks of [8,2]) at the end.

Strategy (memory-regime; weights are ~358 MB fp32 dominated by levels 8-10):
  - Levels 5..9 run in bf16, level 10 in fp8 e3m4 (scale 4, folded into
    fea_dense); activations bf16 from level 5 on, fp32 PSUM accumulation.
    Measured rel err vs the fp32 reference: ~1.4e-2 (threshold 2e-2).
  - Levels in..8 are replicated on all 8 cores; levels 9/10 shard the OUTPUT
    channels across the cores (1/8 of the dominant weight traffic per core).
  - The x9 shard exchange (each core's 512 channels -> all cores) is done
    with direct peer-to-peer SWDGE remote DMAs (XOR slot layout: slot d on
    core r holds the shard of core r^d), NOT an NRT collective: the NRT
    AllGather costs ~40 us of latency; the raw remote DMA exchange costs a
    few us. A compile-time-inserted prelude AllGather (bir_kernel_barrier)
    makes the cross-core SBUF writes safe. Per-slot semaphores let level 10
    consume each peer shard as it lands (slot-pipelined contraction).
  - Level 10's weight chunks are pre-permuted per core into slot order, so
    the contraction walks slots 0..7 (own shard first, no wait).
  - DMA queue split: the big weight stream (wmid/w9/w10) runs on the sync
    HWDGE queue with enough pool buffers that it never stalls; w8 (whose
    pool WAR-waits on L8 compute) runs on the scalar HWDGE queue so it
    cannot head-of-line-block the stream.
  - Final block einsum on the vector engine (broadcast mult + grouped
    reduce); output layout [B, o, block], transposed on host.

kernel(**inputs) takes the FULL unsharded inputs and returns the FULL output.
"""

import ml_dtypes
import numpy as np

NCORES = 8
B = 16
P = 128
C = 8
NLVL = 10
BF16 = ml_dtypes.bfloat16
E3M4 = ml_dtypes.float8_e3m4
W10_SCALE = 4.0  # f10 quantized as e3m4(4*w); 1/4 folded into fea_dense
GATHER = "cc"  # "remote" (peer DMA exchange) or "cc" (NRT AllGather)
# slot d on core r holds the x9 shard of core PERM[r][d] = r ^ PSI[d]; PSI
# measured empirically with a rank-broadcast probe kernel (the driver's
# logical->physical nc remap makes the relative-XOR addressing land slots
# 4..7 with bit 1 flipped: cross-die first hops swap the RMTV lane pair)
PSI = [0, 1, 2, 3, 6, 7, 4, 5]
PERM = [[r ^ PSI[d] for d in range(NCORES)] for r in range(NCORES)]

_CACHE = {}


# ---------------------------------------------------------------- host prep

def _host_prep(inputs):
    """Build the per-core input maps (numpy only)."""
    ind = np.ascontiguousarray(np.asarray(inputs["in_data"], np.float32))
    f = {l: np.asarray(inputs[f"f{l}"], np.float32) for l in range(1, NLVL + 1)}
    f0 = np.asarray(inputs["in_filter"], np.float32)     # [2, 1, 8]
    fd = np.asarray(inputs["fea_dense"], np.float32)     # [1024, 8, 2]

    shared = {}
    # r0 [32, 64, 16]: r0[row, wHi, b] = in[b, wHi*32 + row]
    shared["r0"] = np.ascontiguousarray(
        ind[:, :, 0].reshape(B, 64, 32).transpose(2, 1, 0))

    # w0 [32, 128]: rows (2*wsub + k), cols (wsub*8 + co)
    w0 = np.zeros((32, 128), np.float32)
    for wsub in range(16):
        for k in range(2):
            w0[2 * wsub + k, wsub * 8:wsub * 8 + 8] = f0[k, 0, :]
    shared["w0"] = w0

    # packed levels 1..4 stacked: wpk [4, 128, 128]
    wpk = np.zeros((4, 128, 128), np.float32)
    for lvl in range(1, 5):
        cin = 2 ** (lvl - 1) * C
        cout = 2 ** lvl * C
        s_out = (128 // cin) // 2
        for wso in range(s_out):
            for k in range(2):
                wsi = 2 * wso + k
                wpk[lvl - 1, wsi * cin:(wsi + 1) * cin,
                    wso * cout:(wso + 1) * cout] = f[lvl][k]
    shared["wpk"] = wpk

    # w5/w6/w7 mega-packed [128, 10752] bf16 (kt-major per level), one DMA
    w5v = f[5].astype(BF16).reshape(2, 1, 128, 256)
    w6v = f[6].astype(BF16).reshape(2, 2, 128, 512)
    w7v = f[7].astype(BF16).reshape(2, 4, 128, 1024)
    shared["wmid"] = np.ascontiguousarray(np.concatenate([
        w5v.transpose(2, 0, 1, 3).reshape(128, 512),
        w6v.transpose(2, 0, 1, 3).reshape(128, 2048),
        w7v.transpose(2, 0, 1, 3).reshape(128, 8192)], axis=1))

    # f8/f9/f10 output-channel shards, packed into 4-ci-tile chunks:
    # [nchunks, 128, 4, cout_shard]; chunk m = k*(cin//512) + q, cit = q*4+j
    def shard_pack(fl, cin, dt=BF16):
        sh = fl.shape[-1] // NCORES
        out = []
        flb = fl.astype(dt)
        for r in range(NCORES):
            blk = flb[:, :, r * sh:(r + 1) * sh]
            v = blk.reshape(2, cin // 512, 4, 128, sh).transpose(0, 1, 3, 2, 4)
            out.append(np.ascontiguousarray(
                v.reshape(2 * (cin // 512), 128, 4, sh)))
        return out

    # f8 is REPLICATED (cheaper than a second exchange): co-major chunks
    # [4, 128, kt=16, co=512], kt = k*8 + cit
    f8b = f[8].astype(BF16)
    w8full = np.stack([
        np.ascontiguousarray(
            f8b[:, :, c * 512:(c + 1) * 512]
            .reshape(2, 8, 128, 512).transpose(2, 0, 1, 3).reshape(128, 16, 512))
        for c in range(4)])
    shared["w8"] = w8full

    w9s = shard_pack(f[9], 2048)                      # 8 chunks of [128, 4, 512]
    w10s = shard_pack(f[10] * W10_SCALE, 4096, E3M4)  # 16 chunks of [128, 4, 1024]
    # reorder per core into slot order: new[2d+k] = old[k*8 + slot_rank(d)];
    # in "cc" mode slot d of the gathered x9sb simply holds rank d
    perm = PERM if GATHER == "remote" else [list(range(NCORES))] * NCORES
    w10s = [np.ascontiguousarray(
                w[[k * 8 + perm[r][d] for d in range(8) for k in range(2)]])
            for r, w in enumerate(w10s)]

    # fea_dense shard, per-o flattened, tiled over the 16 batch partitions;
    # carries the 1/W10_SCALE compensation for the e3m4 level-10 weights
    fds = []
    for r in range(NCORES):
        blk = fd[r * 128:(r + 1) * 128] / W10_SCALE        # [128, 8, 2]
        flat = blk.transpose(2, 0, 1).reshape(2, 1024)     # [o, 1024]
        fds.append(np.ascontiguousarray(
            np.broadcast_to(flat[None], (B, 2, 1024))))

    in_maps = []
    for r in range(NCORES):
        m = dict(shared)
        m["w9"] = w9s[r]
        m["w10"] = w10s[r]
        m["fdt"] = fds[r]
        in_maps.append(m)
    return in_maps


# ---------------------------------------------------------------- bass build

def _build():
    import concourse.bass as bass
    import concourse.mybir as mybir
    import concourse.tile as tile
    from concourse import bacc, library_config

    f32 = mybir.dt.float32
    bf16 = mybir.dt.bfloat16
    fp8 = mybir.dt.float8e3
    RELU = mybir.ActivationFunctionType.Relu

    nc = bacc.Bacc("TRN2", target_bir_lowering=False, debug=False,
                   num_devices=NCORES)

    def inp(name, shape, dt=f32):
        return nc.dram_tensor(name, shape, dt, kind="ExternalInput").ap()

    r0 = inp("r0", [32, 64, 16])
    w0 = inp("w0", [32, 128])
    wpk = inp("wpk", [4, 128, 128])
    wmid = inp("wmid", [128, 10752], bf16)
    w8 = inp("w8", [4, 128, 16, 512], bf16)
    w9 = inp("w9", [8, 128, 4, 512], bf16)
    w10 = inp("w10", [16, 128, 4, 1024], fp8)
    fdt = inp("fdt", [B, 2, 1024])
    out = nc.dram_tensor("out", [B, 2, 128], f32, kind="ExternalOutput").ap()

    remote = GATHER == "remote"

    with tile.TileContext(nc) as tc:
        with (
            tc.tile_pool(name="const", bufs=1) as constp,
            tc.tile_pool(name="actp", bufs=3) as actp,
            tc.tile_pool(name="bigp", bufs=1) as bigp,
            tc.tile_pool(name="w7p", bufs=1) as w7p,
            tc.tile_pool(name="w8p", bufs=2) as w8p,
            tc.tile_pool(name="w9p", bufs=8) as w9p,
            tc.tile_pool(name="w10p", bufs=14) as w10p,
            tc.tile_pool(name="psA", bufs=2, space="PSUM") as psA,
            tc.tile_pool(name="psB", bufs=4, space="PSUM") as psB,
            tc.tile_pool(name="psC", bufs=2, space="PSUM") as psC,
            tc.tile_pool(name="dramp", bufs=1, space="DRAM") as dramp,
        ):
            if remote:
                rsems = [nc.alloc_semaphore(f"xch{d}") for d in range(1, NCORES)]
                lsem = nc.alloc_semaphore("xch_local")
                x9rdy = nc.alloc_semaphore("x9rdy")
                nc.gpsimd.load_library(library_config.remote_dma)
                slot_mms = {d: [] for d in range(1, NCORES)}

            if remote:
                # exchange buffers allocated up front so the SWDGE descriptor
                # generation (slow: ~1.7us per broadcast prep on Q7) can run
                # during the initial weight streaming; the data dependency on
                # x9loc defers to the trigger
                x9loc = bigp.tile([128, 4, 2, 16], bf16, name="x9loc")
                x9sb = bigp.tile([128, 32, 2, 16], bf16, name="x9sb")
                preps = []
                for d in range(1, NCORES):
                    rdests = [None] * NCORES
                    rdests[d] = (0, d)
                    preps.append(nc.gpsimd.remote_dma_broadcast(
                        x9sb[:, 4 * d:4 * d + 4], x9loc[:],
                        remote_sem=rsems[d - 1], local_sem=lsem,
                        rdests=rdests))

            # ---- resident loads (sync queue = the main weight stream)
            r0sb = constp.tile([32, 64, 16], f32, name="r0sb")
            nc.sync.dma_start(r0sb[:], r0)
            w0sb = constp.tile([32, 128], f32, name="w0sb")
            nc.sync.dma_start(w0sb[:], w0)
            wpksb = constp.tile([128, 4, 128], f32, name="wpksb")
            nc.sync.dma_start(wpksb[:], wpk.rearrange("l p c -> p l c"))
            wmidsb = w7p.tile([128, 10752], bf16, name="wmidsb")
            nc.sync.dma_start(wmidsb[:], wmid)
            w5sb = wmidsb[:, 0:512].rearrange("p (t c) -> p t c", c=256)
            w6sb = wmidsb[:, 512:2560].rearrange("p (t c) -> p t c", c=512)
            w7sb = wmidsb[:, 2560:10752].rearrange("p (t c) -> p t c", c=1024)
            fdsb = constp.tile([B, 2, 1024], f32, name="fdsb")
            nc.scalar.dma_start(fdsb[:], fdt)

            # ---- input conv + packed levels 1..4 (all [128, 64, 16])
            xprev = None
            for lvl in range(5):
                # x4 feeds the bf16 level-5 matmul, so cast at the relu
                xn = actp.tile([128, 64, 16], bf16 if lvl == 4 else f32,
                               name=f"x{lvl}", tag="xl")
                for ch in range(2):
                    ps = psA.tile([128, 32, 16], f32, name="psA", tag="psA")
                    if lvl == 0:
                        nc.tensor.matmul(
                            ps[:], w0sb[:], r0sb[:, ch * 32:(ch + 1) * 32, :],
                            start=True, stop=True)
                    else:
                        nc.tensor.matmul(
                            ps[:], wpksb[:, lvl - 1, :],
                            xprev[:, ch * 32:(ch + 1) * 32, :],
                            start=True, stop=True)
                    nc.scalar.activation(
                        xn[:, ch * 32:(ch + 1) * 32, :], ps[:], RELU)
                xprev = xn

            # ---- standard levels (orientation A, weights stationary)
            def std_level(xin, wsb, cin_t, cout_t, w_out, name, out_tile=None):
                # xin [128, cin_t, 2*w_out, 16]; wsb [128, 2*cin_t, co] with
                # kt = k*cin_t + cit; returns [128, cout_t, w_out, 16]
                if out_tile is None:
                    xn = actp.tile([128, cout_t, w_out, 16], bf16,
                                   name=name, tag="xl")
                else:
                    xn = out_tile
                for ct in range(cout_t):
                    ps = psA.tile([128, w_out, 16], f32, name="psA", tag="psA")
                    for cit in range(cin_t):
                        rhs2 = xin[:, cit].rearrange(
                            "p (w two) b -> p two w b", two=2)
                        for k in range(2):
                            nc.tensor.matmul(
                                ps[:],
                                wsb[:, k * cin_t + cit,
                                    ct * 128:(ct + 1) * 128],
                                rhs2[:, k],
                                start=(cit == 0 and k == 0),
                                stop=(cit == cin_t - 1 and k == 1))
                    nc.scalar.activation(xn[:, ct], ps[:], RELU)
                return xn

            x5 = std_level(xprev[:, None], w5sb, 1, 2, 32, "x5")
            x6 = std_level(x5, w6sb, 2, 4, 16, "x6")
            x7 = std_level(x6, w7sb, 4, 8, 8, "x7")

            # ---- level 8 REPLICATED (full 2048 cout), co-major weight stream
            # w8 chunks stream on the scalar queue: their pool WAR-waits on L8
            # compute and must not block the sync queue's w9/w10 stream.
            x8sb = bigp.tile([128, 16, 4, 16], bf16, name="x8sb")
            for c in range(4):
                w8c = w8p.tile([128, 16, 512], bf16, name="w8c", tag="w8c")
                nc.scalar.dma_start(w8c[:], w8[c])
                for ctl in range(4):
                    ps = psA.tile([128, 4, 16], f32, name="psA", tag="psA")
                    for cit in range(8):
                        rhs2 = x7[:, cit].rearrange(
                            "p (w two) b -> p two w b", two=2)
                        for k in range(2):
                            nc.tensor.matmul(
                                ps[:],
                                w8c[:, k * 8 + cit, ctl * 128:(ctl + 1) * 128],
                                rhs2[:, k],
                                start=(cit == 0 and k == 0),
                                stop=(cit == 7 and k == 1))
                    nc.scalar.activation(x8sb[:, c * 4 + ctl], ps[:], RELU)

            # ---- level 9 (512-ch shard, streamed weights, 4 accumulators)
            ps9 = [psB.tile([128, 2, 16], f32, name=f"ps9_{ct}", tag="psB")
                   for ct in range(4)]
            for m in range(8):
                w9c = w9p.tile([128, 4, 512], bf16, name="w9c", tag="w9c")
                nc.sync.dma_start(w9c[:], w9[m])
                k, q = divmod(m, 4)
                for j in range(4):
                    cit = q * 4 + j
                    rhs = x8sb[:, cit].rearrange(
                        "p (w two) b -> p two w b", two=2)[:, k]
                    for ct in range(4):
                        nc.tensor.matmul(
                            ps9[ct][:],
                            w9c[:, j, ct * 128:(ct + 1) * 128],
                            rhs,
                            start=(m == 0 and j == 0),
                            stop=(m == 7 and j == 3))
            if not remote:
                x9loc = bigp.tile([128, 4, 2, 16], bf16, name="x9loc")
            for ct in range(4):
                nc.scalar.activation(x9loc[:, ct], ps9[ct][:], RELU)
            if remote:
                # The early-emitted SWDGE preps had no x9loc producer to
                # defer a RAW edge from, so trigger1 would fire before x9loc
                # exists. A reader spanning all four relu outputs bumps an
                # explicit readiness semaphore for it instead (the relus'
                # own sync-update slots are taken).
                x9scr = bigp.tile([128, 4, 16], bf16, name="x9scr")
                x9cp = nc.vector.tensor_copy(x9scr[:], x9loc[:, :, 1, :])

            # ---- x9 shard exchange: push my shard into slot d of peer
            # me^d; peer's slot d gets +2 on rsems[d-1] when the bytes have
            # landed. A second redundant send round (same bytes, same sems)
            # covers the race where a late-starting peer's start-of-kernel
            # sem clear wipes the first round's increments.
            if remote:
                trig1 = nc.gpsimd.trigger_dma(count=None)
                for d in range(1, NCORES):
                    rdests = [None] * NCORES
                    rdests[d] = (0, d)
                    nc.gpsimd.remote_dma_broadcast(
                        x9sb[:, 4 * d:4 * d + 4], x9loc[:],
                        remote_sem=rsems[d - 1], local_sem=lsem,
                        rdests=rdests)
                nc.gpsimd.trigger_dma(count=None)
            else:
                x9sb = bigp.tile([128, 32, 2, 16], bf16, name="x9sb")
                ag9_in = dramp.tile([1, 128, 4, 2, 16], bf16, name="ag9_in")
                ag9_out = dramp.tile([NCORES, 128, 4, 2, 16], bf16,
                                     name="ag9_out", addr_space="Shared")
                nc.scalar.dma_start(ag9_in[0], x9loc[:])
                nc.gpsimd.collective_compute(
                    "AllGather", mybir.AluOpType.bypass,
                    replica_groups=[list(range(NCORES))],
                    ins=[ag9_in.opt()], outs=[ag9_out.opt()])
                for r in range(NCORES):
                    nc.scalar.dma_start(x9sb[:, 4 * r:4 * r + 4], ag9_out[r])

            # ---- level 10 (1024-ch shard, orientation B: acts stationary,
            #      e3m4 weights moving, slot-pipelined over peer shards)
            ps10 = [psC.tile([B, 512], f32, name=f"ps10_{cb}", tag="psC")
                    for cb in range(2)]
            for d in range(NCORES):
                if remote and d == 0:
                    src = x9loc
                    base = 0
                else:
                    src = x9sb
                    base = 4 * d
                for k in range(2):
                    m = 2 * d + k
                    w10c = w10p.tile([128, 4, 1024], fp8, name="w10c",
                                     tag="w10c")
                    nc.sync.dma_start(w10c[:], w10[m])
                    for j in range(4):
                        lhsT = src[:, base + j, k, :]
                        for cb in range(2):
                            mm = nc.tensor.matmul(
                                ps10[cb][:], lhsT,
                                w10c[:, j, cb * 512:(cb + 1) * 512],
                                start=(d == 0 and k == 0 and j == 0),
                                stop=(d == NCORES - 1 and k == 1 and j == 3))
                            if remote and d > 0 and k == 0 and j == 0:
                                # first matmul of each psum chain that touches
                                # slot d; gets the arrival wait attached
                                # post-scheduling (invisible to the tile sim)
                                slot_mms[d].append(mm)
            x10 = bigp.tile([B, 1024], f32, name="x10")
            for cb in range(2):
                nc.scalar.activation(
                    x10[:, cb * 512:(cb + 1) * 512], ps10[cb][:], RELU)

            # ---- final per-block einsum on the vector engine
            # osb laid out [B, o, k]; host transposes to [B, k, o]
            prod = bigp.tile([B, 2, 1024], f32, name="prod")
            osb = bigp.tile([B, 2, 128], f32, name="osb")
            x10b = x10.rearrange("p (one f) -> p one f", one=1).broadcast_to(
                [B, 2, 1024])
            nc.vector.tensor_tensor(
                prod[:], x10b, fdsb[:], mybir.AluOpType.mult)
            nc.vector.tensor_reduce(
                osb[:],
                prod.rearrange("p o (k c) -> p o k c", c=8),
                mybir.AxisListType.X, mybir.AluOpType.add)
            nc.scalar.dma_start(out, osb[:])

    if remote:
        # Cross-core waits are attached AFTER the tile block: the tile
        # scheduling pass simulates a single core and would report the
        # remote-satisfied semaphores as a deadlock.
        #  - a PE-stream wait right before the first matmul touching slot d
        #    gates the whole slot on the peer shard having landed (+2 on
        #    rsems[d-1]); inserted directly into the lowered instruction
        #    list (the matmuls' own wait slots are already taken by the
        #    tile-assigned weight-chunk arrival waits)
        def find_block(ins):
            for blk in nc.main_func.blocks:
                for i, x in enumerate(blk.instructions):
                    if x.name == ins.name:
                        return blk, i
            raise KeyError(ins.name)

        for d in range(1, NCORES):
            w = nc.tensor.wait_ge(rsems[d - 1], 2).ins
            wblk, widx = find_block(w)
            del wblk.instructions[widx]
            # insert before whichever of the slot's chain-first matmuls the
            # scheduler placed earliest
            places = [find_block(mm.ins) for mm in slot_mms[d]]
            mblk, midx = min(places, key=lambda bi: bi[1])
            assert all(b is mblk for b, _ in places)
            mblk.instructions.insert(midx, w)
        #  - trigger1 must not fire descriptors before x9loc is computed.
        #    The x9scr copy reads all four relu outputs; a nop-with-update
        #    placed right after it on the in-order vector stream bumps x9rdy
        #    (no instruction has a free sync-update slot for a second
        #    update, so the increment needs its own instruction)
        ni = nc.vector.nop(nofuse=True).then_inc(x9rdy, 1)
        niblk, niidx = find_block(ni.ins)
        del niblk.instructions[niidx]
        cblk, cidx = find_block(x9cp.ins)
        cblk.instructions.insert(cidx + 1, ni.ins)
        wt = nc.gpsimd.wait_ge(x9rdy, 1).ins
        wblk, widx = find_block(wt)
        del wblk.instructions[widx]
        tblk, tidx = find_block(trig1.ins)
        tblk.instructions.insert(tidx, wt)
        # Register the kernel-entry barrier replica group so compile()
        # inserts its prelude AllGather. NOTHING in the program waits on it:
        # it exists purely so the NEFF contains a collective, which makes
        # the runtime rendezvous the 8 cores' execution starts (without it,
        # profiler arming staggers the starts by milliseconds and every
        # early core idle-waits that long for peer shards).
        nc._bir_kernel_barrier_sem_replica_groups.append(set(range(NCORES)))
        # End-of-kernel drains: hold the epilogue semaphore zeroing until
        # every in-flight increment has landed -- both rounds of every peer's
        # push to me (sem_d reaches 4) and my own sends' local ticks. This
        # makes the between-executions state clean for any launch skew: once
        # a semaphore reads its final value nothing else is in flight, so
        # the epilogue clear cannot lose increments.
        for d in range(1, NCORES):
            nc.gpsimd.wait_ge(rsems[d - 1], 4)
        nc.gpsimd.wait_ge(lsem, 2 * 16 * (NCORES - 1))

    nc.compile()
    return nc


def _build_cleaner():
    """A trivial one-shot NEFF that zeroes the kernel semaphore range.

    The main kernel's cross-core exchange assumes its semaphores start at 0.
    Executions N>=1 get that from execution N-1's epilogue clear (held back
    by the end-of-kernel drains until all increments landed), but the very
    first execution after NEFF load would see whatever junk previous NEFFs
    left in the semaphore file. Running this cleaner once before the first
    real execution makes execution 0 sound too.
    """
    import concourse.mybir as mybir
    import concourse.tile as tile
    from concourse import bacc

    f32 = mybir.dt.float32
    nc = bacc.Bacc("TRN2", target_bir_lowering=False, debug=False,
                   num_devices=NCORES)
    out = nc.dram_tensor("out", [1, 128], f32, kind="ExternalOutput")
    with tile.TileContext(nc) as tc:
        with tc.tile_pool(name="p", bufs=1) as pool:
            z = pool.tile([1, 128], f32, name="z")
            nc.vector.memset(z[:], 0.0)
            nc.sync.dma_start(out.ap(), z[:])
    c = nc.gpsimd.sem_clear(range(100, 255)).ins
    for blk in nc.main_func.blocks:
        for i, x in enumerate(blk.instructions):
            if x.name == c.name:
                del blk.instructions[i]
                break
    tgt = next(b for b in nc.main_func.blocks
               if "tile_context" in b.name and "end" not in b.name)
    tgt.instructions.insert(0, c)
    nc.compile()
    return nc


# ------------------------------------------------------------------- kernel

def kernel(**inputs):
    from concourse.bass_utils import run_bass_kernel_spmd

    in_maps = _host_prep(inputs)
    if "nc" not in _CACHE:
        _CACHE["nc"] = _build()
        if GATHER == "remote":
            # zero the semaphore file (previous NEFFs leave junk), then run
            # one discarded warmup execution: the very first execution after
            # NEFF load delivers remote SBUF writes unreliably (sem
            # increments arrive, some data lanes don't), so its output is
            # garbage; every later execution is sound.
            cl = _build_cleaner()
            run_bass_kernel_spmd(cl, [{} for _ in range(NCORES)],
                                 core_ids=list(range(NCORES)))
            run_bass_kernel_spmd(_CACHE["nc"], in_maps,
                                 core_ids=list(range(NCORES)))
    nc = _CACHE["nc"]
    res = run_bass_kernel_spmd(nc, in_maps, core_ids=list(range(NCORES)))
    parts = [res.results[r]["out"].transpose(0, 2, 1)       # [16, 128, 2]
             for r in range(NCORES)]
    full = np.concatenate(parts, axis=1)                    # [16, 1024, 2]
    return np.ascontiguousarray(full.reshape(B, 2048, 1).astype(np.float32))


# revision 45
# speedup vs baseline: 1.1317x; 1.0699x over previous
"""Trainium2 Bass kernel for the butterfly-CNN problem (nn_CNNLayer_30296699306356).

Network (see problem reference): input conv (k=2,s=2, 1->8 ch) + 10 butterfly
conv levels (k=2,s=2, channels double each level, relu, zero biases) + a
per-block dense matmul (1024 blocks of [8,2]) at the end.

Strategy (memory-regime; weights are ~358 MB fp32 dominated by levels 8-10):
  - Levels 5..9 run in bf16, level 10 in fp8 e3m4 (scale 4, folded into
    fea_dense); activations bf16 from level 5 on, fp32 PSUM accumulation.
    Measured rel err vs the fp32 reference: ~1.4e-2 (threshold 2e-2).
  - Levels in..8 are replicated on all 8 cores; levels 9/10 shard the OUTPUT
    channels across the cores (1/8 of the dominant weight traffic per core).
  - The x9 shard exchange (each core's 512 channels -> all cores) is done
    with direct peer-to-peer SWDGE remote DMAs (XOR slot layout: slot d on
    core r holds the shard of core r^d), NOT an NRT collective: the NRT
    AllGather costs ~40 us of latency; the raw remote DMA exchange costs a
    few us. A compile-time-inserted prelude AllGather (bir_kernel_barrier)
    makes the cross-core SBUF writes safe. Per-slot semaphores let level 10
    consume each peer shard as it lands (slot-pipelined contraction).
  - Level 10's weight chunks are pre-permuted per core into slot order, so
    the contraction walks slots 0..7 (own shard first, no wait).
  - DMA queue split: the big weight stream (wmid/w9/w10) runs on the sync
    HWDGE queue with enough pool buffers that it never stalls; w8 (whose
    pool WAR-waits on L8 compute) runs on the scalar HWDGE queue so it
    cannot head-of-line-block the stream.
  - Final block einsum on the vector engine (broadcast mult + grouped
    reduce); output layout [B, o, block], transposed on host.

kernel(**inputs) takes the FULL unsharded inputs and returns the FULL output.
"""

import ml_dtypes
import numpy as np

NCORES = 8
B = 16
P = 128
C = 8
NLVL = 10
BF16 = ml_dtypes.bfloat16
E3M4 = ml_dtypes.float8_e3m4
W10_SCALE = 4.0  # f10 quantized as e3m4(4*w); 1/4 folded into fea_dense
GATHER = "cc"  # "remote" (peer DMA exchange) or "cc" (NRT AllGather)
# slot d on core r holds the x9 shard of core PERM[r][d] = r ^ PSI[d]; PSI
# measured empirically with a rank-broadcast probe kernel (the driver's
# logical->physical nc remap makes the relative-XOR addressing land slots
# 4..7 with bit 1 flipped: cross-die first hops swap the RMTV lane pair)
PSI = [0, 1, 2, 3, 6, 7, 4, 5]
PERM = [[r ^ PSI[d] for d in range(NCORES)] for r in range(NCORES)]

_CACHE = {}


# ---------------------------------------------------------------- host prep

def _host_prep(inputs):
    """Build the per-core input maps (numpy only)."""
    ind = np.ascontiguousarray(np.asarray(inputs["in_data"], np.float32))
    f = {l: np.asarray(inputs[f"f{l}"], np.float32) for l in range(1, NLVL + 1)}
    f0 = np.asarray(inputs["in_filter"], np.float32)     # [2, 1, 8]
    fd = np.asarray(inputs["fea_dense"], np.float32)     # [1024, 8, 2]

    shared = {}
    # r0 [32, 64, 16]: r0[row, wHi, b] = in[b, wHi*32 + row]
    shared["r0"] = np.ascontiguousarray(
        ind[:, :, 0].reshape(B, 64, 32).transpose(2, 1, 0))

    # w0 [32, 128]: rows (2*wsub + k), cols (wsub*8 + co)
    w0 = np.zeros((32, 128), np.float32)
    for wsub in range(16):
        for k in range(2):
            w0[2 * wsub + k, wsub * 8:wsub * 8 + 8] = f0[k, 0, :]
    shared["w0"] = w0

    # packed levels 1..4 stacked: wpk [4, 128, 128]
    wpk = np.zeros((4, 128, 128), np.float32)
    for lvl in range(1, 5):
        cin = 2 ** (lvl - 1) * C
        cout = 2 ** lvl * C
        s_out = (128 // cin) // 2
        for wso in range(s_out):
            for k in range(2):
                wsi = 2 * wso + k
                wpk[lvl - 1, wsi * cin:(wsi + 1) * cin,
                    wso * cout:(wso + 1) * cout] = f[lvl][k]
    shared["wpk"] = wpk

    # w5/w6/w7 mega-packed [128, 10752] bf16 (kt-major per level), one DMA
    w5v = f[5].astype(BF16).reshape(2, 1, 128, 256)
    w6v = f[6].astype(BF16).reshape(2, 2, 128, 512)
    w7v = f[7].astype(BF16).reshape(2, 4, 128, 1024)
    shared["wmid"] = np.ascontiguousarray(np.concatenate([
        w5v.transpose(2, 0, 1, 3).reshape(128, 512),
        w6v.transpose(2, 0, 1, 3).reshape(128, 2048),
        w7v.transpose(2, 0, 1, 3).reshape(128, 8192)], axis=1))

    # f8/f9/f10 output-channel shards, packed into 4-ci-tile chunks:
    # [nchunks, 128, 4, cout_shard]; chunk m = k*(cin//512) + q, cit = q*4+j
    def shard_pack(fl, cin, dt=BF16):
        sh = fl.shape[-1] // NCORES
        out = []
        flb = fl.astype(dt)
        for r in range(NCORES):
            blk = flb[:, :, r * sh:(r + 1) * sh]
            v = blk.reshape(2, cin // 512, 4, 128, sh).transpose(0, 1, 3, 2, 4)
            out.append(np.ascontiguousarray(
                v.reshape(2 * (cin // 512), 128, 4, sh)))
        return out

    # f8 is REPLICATED (cheaper than a second exchange): co-major chunks
    # [4, 128, kt=16, co=512], kt = k*8 + cit
    f8b = f[8].astype(BF16)
    w8full = np.stack([
        np.ascontiguousarray(
            f8b[:, :, c * 512:(c + 1) * 512]
            .reshape(2, 8, 128, 512).transpose(2, 0, 1, 3).reshape(128, 16, 512))
        for c in range(4)])
    shared["w8"] = w8full

    w9s = shard_pack(f[9], 2048)                      # 8 chunks of [128, 4, 512]
    w10s = shard_pack(f[10] * W10_SCALE, 4096, E3M4)  # 16 chunks of [128, 4, 1024]
    # reorder per core into slot order: new[2d+k] = old[k*8 + slot_rank(d)];
    # in "cc" mode slot d of the gathered x9sb simply holds rank d
    perm = PERM if GATHER == "remote" else [list(range(NCORES))] * NCORES
    w10s = [np.ascontiguousarray(
                w[[k * 8 + perm[r][d] for d in range(8) for k in range(2)]])
            for r, w in enumerate(w10s)]

    # fea_dense shard, per-o flattened, tiled over the 16 batch partitions;
    # carries the 1/W10_SCALE compensation for the e3m4 level-10 weights
    fds = []
    for r in range(NCORES):
        blk = fd[r * 128:(r + 1) * 128] / W10_SCALE        # [128, 8, 2]
        flat = blk.transpose(2, 0, 1).reshape(2, 1024)     # [o, 1024]
        fds.append(np.ascontiguousarray(
            np.broadcast_to(flat[None], (B, 2, 1024))))

    in_maps = []
    for r in range(NCORES):
        m = dict(shared)
        m["w9"] = w9s[r]
        m["w10"] = w10s[r]
        m["fdt"] = fds[r]
        in_maps.append(m)
    return in_maps


# ---------------------------------------------------------------- bass build

def _build():
    import concourse.bass as bass
    import concourse.mybir as mybir
    import concourse.tile as tile
    from concourse import bacc, library_config

    f32 = mybir.dt.float32
    bf16 = mybir.dt.bfloat16
    fp8 = mybir.dt.float8e3
    RELU = mybir.ActivationFunctionType.Relu

    nc = bacc.Bacc("TRN2", target_bir_lowering=False, debug=False,
                   num_devices=NCORES)

    def inp(name, shape, dt=f32):
        return nc.dram_tensor(name, shape, dt, kind="ExternalInput").ap()

    r0 = inp("r0", [32, 64, 16])
    w0 = inp("w0", [32, 128])
    wpk = inp("wpk", [4, 128, 128])
    wmid = inp("wmid", [128, 10752], bf16)
    w8 = inp("w8", [4, 128, 16, 512], bf16)
    w9 = inp("w9", [8, 128, 4, 512], bf16)
    w10 = inp("w10", [16, 128, 4, 1024], fp8)
    fdt = inp("fdt", [B, 2, 1024])
    out = nc.dram_tensor("out", [B, 2, 128], f32, kind="ExternalOutput").ap()

    remote = GATHER == "remote"

    with tile.TileContext(nc) as tc:
        with (
            tc.tile_pool(name="const", bufs=1) as constp,
            tc.tile_pool(name="actp", bufs=3) as actp,
            tc.tile_pool(name="bigp", bufs=1) as bigp,
            tc.tile_pool(name="w7p", bufs=1) as w7p,
            tc.tile_pool(name="w8p", bufs=2) as w8p,
            tc.tile_pool(name="w9p", bufs=8) as w9p,
            tc.tile_pool(name="w10p", bufs=14) as w10p,
            tc.tile_pool(name="psA", bufs=2, space="PSUM") as psA,
            tc.tile_pool(name="psB", bufs=4, space="PSUM") as psB,
            tc.tile_pool(name="psC", bufs=2, space="PSUM") as psC,
            tc.tile_pool(name="dramp", bufs=1, space="DRAM") as dramp,
        ):
            if remote:
                rsems = [nc.alloc_semaphore(f"xch{d}") for d in range(1, NCORES)]
                lsem = nc.alloc_semaphore("xch_local")
                x9rdy = nc.alloc_semaphore("x9rdy")
                nc.gpsimd.load_library(library_config.remote_dma)
                slot_mms = {d: [] for d in range(1, NCORES)}

            if remote:
                # exchange buffers allocated up front so the SWDGE descriptor
                # generation (slow: ~1.7us per broadcast prep on Q7) can run
                # during the initial weight streaming; the data dependency on
                # x9loc defers to the trigger
                x9loc = bigp.tile([128, 4, 2, 16], bf16, name="x9loc")
                x9sb = bigp.tile([128, 32, 2, 16], bf16, name="x9sb")
                preps = []
                for d in range(1, NCORES):
                    rdests = [None] * NCORES
                    rdests[d] = (0, d)
                    preps.append(nc.gpsimd.remote_dma_broadcast(
                        x9sb[:, 4 * d:4 * d + 4], x9loc[:],
                        remote_sem=rsems[d - 1], local_sem=lsem,
                        rdests=rdests))

            # ---- resident loads (sync queue = the main weight stream)
            r0sb = constp.tile([32, 64, 16], f32, name="r0sb")
            nc.sync.dma_start(r0sb[:], r0)
            w0sb = constp.tile([32, 128], f32, name="w0sb")
            nc.sync.dma_start(w0sb[:], w0)
            wpksb = constp.tile([128, 4, 128], f32, name="wpksb")
            nc.sync.dma_start(wpksb[:], wpk.rearrange("l p c -> p l c"))
            wmidsb = w7p.tile([128, 10752], bf16, name="wmidsb")
            nc.sync.dma_start(wmidsb[:], wmid)
            w5sb = wmidsb[:, 0:512].rearrange("p (t c) -> p t c", c=256)
            w6sb = wmidsb[:, 512:2560].rearrange("p (t c) -> p t c", c=512)
            w7sb = wmidsb[:, 2560:10752].rearrange("p (t c) -> p t c", c=1024)
            fdsb = constp.tile([B, 2, 1024], f32, name="fdsb")
            nc.scalar.dma_start(fdsb[:], fdt)

            # ---- input conv + packed levels 1..4 (all [128, 64, 16])
            xprev = None
            for lvl in range(5):
                # x4 feeds the bf16 level-5 matmul, so cast at the relu
                xn = actp.tile([128, 64, 16], bf16 if lvl == 4 else f32,
                               name=f"x{lvl}", tag="xl")
                for ch in range(2):
                    ps = psA.tile([128, 32, 16], f32, name="psA", tag="psA")
                    if lvl == 0:
                        nc.tensor.matmul(
                            ps[:], w0sb[:], r0sb[:, ch * 32:(ch + 1) * 32, :],
                            start=True, stop=True)
                    else:
                        nc.tensor.matmul(
                            ps[:], wpksb[:, lvl - 1, :],
                            xprev[:, ch * 32:(ch + 1) * 32, :],
                            start=True, stop=True)
                    nc.scalar.activation(
                        xn[:, ch * 32:(ch + 1) * 32, :], ps[:], RELU)
                xprev = xn

            # ---- standard levels (orientation A, weights stationary)
            def std_level(xin, wsb, cin_t, cout_t, w_out, name, out_tile=None):
                # xin [128, cin_t, 2*w_out, 16]; wsb [128, 2*cin_t, co] with
                # kt = k*cin_t + cit; returns [128, cout_t, w_out, 16]
                if out_tile is None:
                    xn = actp.tile([128, cout_t, w_out, 16], bf16,
                                   name=name, tag="xl")
                else:
                    xn = out_tile
                for ct in range(cout_t):
                    ps = psA.tile([128, w_out, 16], f32, name="psA", tag="psA")
                    for cit in range(cin_t):
                        rhs2 = xin[:, cit].rearrange(
                            "p (w two) b -> p two w b", two=2)
                        for k in range(2):
                            nc.tensor.matmul(
                                ps[:],
                                wsb[:, k * cin_t + cit,
                                    ct * 128:(ct + 1) * 128],
                                rhs2[:, k],
                                start=(cit == 0 and k == 0),
                                stop=(cit == cin_t - 1 and k == 1))
                    nc.scalar.activation(xn[:, ct], ps[:], RELU)
                return xn

            x5 = std_level(xprev[:, None], w5sb, 1, 2, 32, "x5")
            x6 = std_level(x5, w6sb, 2, 4, 16, "x6")
            x7 = std_level(x6, w7sb, 4, 8, 8, "x7")

            # ---- level 8 REPLICATED (full 2048 cout), co-major weight stream
            # w8 chunks stream on the scalar queue: their pool WAR-waits on L8
            # compute and must not block the sync queue's w9/w10 stream.
            x8sb = bigp.tile([128, 16, 4, 16], bf16, name="x8sb")
            for c in range(4):
                w8c = w8p.tile([128, 16, 512], bf16, name="w8c", tag="w8c")
                nc.sync.dma_start(w8c[:], w8[c])
                for ctl in range(4):
                    ps = psA.tile([128, 4, 16], f32, name="psA", tag="psA")
                    for cit in range(8):
                        rhs2 = x7[:, cit].rearrange(
                            "p (w two) b -> p two w b", two=2)
                        for k in range(2):
                            nc.tensor.matmul(
                                ps[:],
                                w8c[:, k * 8 + cit, ctl * 128:(ctl + 1) * 128],
                                rhs2[:, k],
                                start=(cit == 0 and k == 0),
                                stop=(cit == 7 and k == 1))
                    nc.scalar.activation(x8sb[:, c * 4 + ctl], ps[:], RELU)

            # ---- level 9 (512-ch shard, streamed weights, 4 accumulators)
            ps9 = [psB.tile([128, 2, 16], f32, name=f"ps9_{ct}", tag="psB")
                   for ct in range(4)]
            for m in range(8):
                w9c = w9p.tile([128, 4, 512], bf16, name="w9c", tag="w9c")
                nc.sync.dma_start(w9c[:], w9[m])
                k, q = divmod(m, 4)
                for j in range(4):
                    cit = q * 4 + j
                    rhs = x8sb[:, cit].rearrange(
                        "p (w two) b -> p two w b", two=2)[:, k]
                    for ct in range(4):
                        nc.tensor.matmul(
                            ps9[ct][:],
                            w9c[:, j, ct * 128:(ct + 1) * 128],
                            rhs,
                            start=(m == 0 and j == 0),
                            stop=(m == 7 and j == 3))
            if not remote:
                x9loc = bigp.tile([128, 4, 2, 16], bf16, name="x9loc")
            for ct in range(4):
                nc.scalar.activation(x9loc[:, ct], ps9[ct][:], RELU)
            if remote:
                # The early-emitted SWDGE preps had no x9loc producer to
                # defer a RAW edge from, so trigger1 would fire before x9loc
                # exists. A reader spanning all four relu outputs bumps an
                # explicit readiness semaphore for it instead (the relus'
                # own sync-update slots are taken).
                x9scr = bigp.tile([128, 4, 16], bf16, name="x9scr")
                x9cp = nc.vector.tensor_copy(x9scr[:], x9loc[:, :, 1, :])

            # ---- x9 shard exchange: push my shard into slot d of peer
            # me^d; peer's slot d gets +2 on rsems[d-1] when the bytes have
            # landed. A second redundant send round (same bytes, same sems)
            # covers the race where a late-starting peer's start-of-kernel
            # sem clear wipes the first round's increments.
            if remote:
                trig1 = nc.gpsimd.trigger_dma(count=None)
                for d in range(1, NCORES):
                    rdests = [None] * NCORES
                    rdests[d] = (0, d)
                    nc.gpsimd.remote_dma_broadcast(
                        x9sb[:, 4 * d:4 * d + 4], x9loc[:],
                        remote_sem=rsems[d - 1], local_sem=lsem,
                        rdests=rdests)
                nc.gpsimd.trigger_dma(count=None)
            else:
                x9sb = bigp.tile([128, 32, 2, 16], bf16, name="x9sb")
                ag9_in = dramp.tile([1, 128, 4, 2, 16], bf16, name="ag9_in")
                ag9_out = dramp.tile([NCORES, 128, 4, 2, 16], bf16,
                                     name="ag9_out", addr_space="Shared")
                nc.scalar.dma_start(ag9_in[0], x9loc[:])
                nc.gpsimd.collective_compute(
                    "AllGather", mybir.AluOpType.bypass,
                    replica_groups=[list(range(NCORES))],
                    ins=[ag9_in.opt()], outs=[ag9_out.opt()])
                for r in range(NCORES):
                    nc.scalar.dma_start(x9sb[:, 4 * r:4 * r + 4], ag9_out[r])

            # ---- level 10 (1024-ch shard, orientation B: acts stationary,
            #      e3m4 weights moving, slot-pipelined over peer shards)
            ps10 = [psC.tile([B, 512], f32, name=f"ps10_{cb}", tag="psC")
                    for cb in range(2)]
            for d in range(NCORES):
                if remote and d == 0:
                    src = x9loc
                    base = 0
                else:
                    src = x9sb
                    base = 4 * d
                for k in range(2):
                    m = 2 * d + k
                    w10c = w10p.tile([128, 4, 1024], fp8, name="w10c",
                                     tag="w10c")
                    nc.sync.dma_start(w10c[:], w10[m])
                    for j in range(4):
                        lhsT = src[:, base + j, k, :]
                        for cb in range(2):
                            mm = nc.tensor.matmul(
                                ps10[cb][:], lhsT,
                                w10c[:, j, cb * 512:(cb + 1) * 512],
                                start=(d == 0 and k == 0 and j == 0),
                                stop=(d == NCORES - 1 and k == 1 and j == 3))
                            if remote and d > 0 and k == 0 and j == 0:
                                # first matmul of each psum chain that touches
                                # slot d; gets the arrival wait attached
                                # post-scheduling (invisible to the tile sim)
                                slot_mms[d].append(mm)
            x10 = bigp.tile([B, 1024], f32, name="x10")
            for cb in range(2):
                nc.scalar.activation(
                    x10[:, cb * 512:(cb + 1) * 512], ps10[cb][:], RELU)

            # ---- final per-block einsum on the vector engine
            # osb laid out [B, o, k]; host transposes to [B, k, o]
            prod = bigp.tile([B, 2, 1024], f32, name="prod")
            osb = bigp.tile([B, 2, 128], f32, name="osb")
            x10b = x10.rearrange("p (one f) -> p one f", one=1).broadcast_to(
                [B, 2, 1024])
            nc.vector.tensor_tensor(
                prod[:], x10b, fdsb[:], mybir.AluOpType.mult)
            nc.vector.tensor_reduce(
                osb[:],
                prod.rearrange("p o (k c) -> p o k c", c=8),
                mybir.AxisListType.X, mybir.AluOpType.add)
            nc.scalar.dma_start(out, osb[:])

    if remote:
        # Cross-core waits are attached AFTER the tile block: the tile
        # scheduling pass simulates a single core and would report the
        # remote-satisfied semaphores as a deadlock.
        #  - a PE-stream wait right before the first matmul touching slot d
        #    gates the whole slot on the peer shard having landed (+2 on
        #    rsems[d-1]); inserted directly into the lowered instruction
        #    list (the matmuls' own wait slots are already taken by the
        #    tile-assigned weight-chunk arrival waits)
        def find_block(ins):
            for blk in nc.main_func.blocks:
                for i, x in enumerate(blk.instructions):
                    if x.name == ins.name:
                        return blk, i
            raise KeyError(ins.name)

        for d in range(1, NCORES):
            w = nc.tensor.wait_ge(rsems[d - 1], 2).ins
            wblk, widx = find_block(w)
            del wblk.instructions[widx]
            # insert before whichever of the slot's chain-first matmuls the
            # scheduler placed earliest
            places = [find_block(mm.ins) for mm in slot_mms[d]]
            mblk, midx = min(places, key=lambda bi: bi[1])
            assert all(b is mblk for b, _ in places)
            mblk.instructions.insert(midx, w)
        #  - trigger1 must not fire descriptors before x9loc is computed.
        #    The x9scr copy reads all four relu outputs; a nop-with-update
        #    placed right after it on the in-order vector stream bumps x9rdy
        #    (no instruction has a free sync-update slot for a second
        #    update, so the increment needs its own instruction)
        ni = nc.vector.nop(nofuse=True).then_inc(x9rdy, 1)
        niblk, niidx = find_block(ni.ins)
        del niblk.instructions[niidx]
        cblk, cidx = find_block(x9cp.ins)
        cblk.instructions.insert(cidx + 1, ni.ins)
        wt = nc.gpsimd.wait_ge(x9rdy, 1).ins
        wblk, widx = find_block(wt)
        del wblk.instructions[widx]
        tblk, tidx = find_block(trig1.ins)
        tblk.instructions.insert(tidx, wt)
        # Register the kernel-entry barrier replica group so compile()
        # inserts its prelude AllGather. NOTHING in the program waits on it:
        # it exists purely so the NEFF contains a collective, which makes
        # the runtime rendezvous the 8 cores' execution starts (without it,
        # profiler arming staggers the starts by milliseconds and every
        # early core idle-waits that long for peer shards).
        nc._bir_kernel_barrier_sem_replica_groups.append(set(range(NCORES)))
        # End-of-kernel drains: hold the epilogue semaphore zeroing until
        # every in-flight increment has landed -- both rounds of every peer's
        # push to me (sem_d reaches 4) and my own sends' local ticks. This
        # makes the between-executions state clean for any launch skew: once
        # a semaphore reads its final value nothing else is in flight, so
        # the epilogue clear cannot lose increments.
        for d in range(1, NCORES):
            nc.gpsimd.wait_ge(rsems[d - 1], 4)
        nc.gpsimd.wait_ge(lsem, 2 * 16 * (NCORES - 1))

    nc.compile()
    return nc


def _build_cleaner():
    """A trivial one-shot NEFF that zeroes the kernel semaphore range.

    The main kernel's cross-core exchange assumes its semaphores start at 0.
    Executions N>=1 get that from execution N-1's epilogue clear (held back
    by the end-of-kernel drains until all increments landed), but the very
    first execution after NEFF load would see whatever junk previous NEFFs
    left in the semaphore file. Running this cleaner once before the first
    real execution makes execution 0 sound too.
    """
    import concourse.mybir as mybir
    import concourse.tile as tile
    from concourse import bacc

    f32 = mybir.dt.float32
    nc = bacc.Bacc("TRN2", target_bir_lowering=False, debug=False,
                   num_devices=NCORES)
    out = nc.dram_tensor("out", [1, 128], f32, kind="ExternalOutput")
    with tile.TileContext(nc) as tc:
        with tc.tile_pool(name="p", bufs=1) as pool:
            z = pool.tile([1, 128], f32, name="z")
            nc.vector.memset(z[:], 0.0)
            nc.sync.dma_start(out.ap(), z[:])
    c = nc.gpsimd.sem_clear(range(100, 255)).ins
    for blk in nc.main_func.blocks:
        for i, x in enumerate(blk.instructions):
            if x.name == c.name:
                del blk.instructions[i]
                break
    tgt = next(b for b in nc.main_func.blocks
               if "tile_context" in b.name and "end" not in b.name)
    tgt.instructions.insert(0, c)
    nc.compile()
    return nc


# ------------------------------------------------------------------- kernel

def kernel(**inputs):
    from concourse.bass_utils import run_bass_kernel_spmd

    in_maps = _host_prep(inputs)
    if "nc" not in _CACHE:
        _CACHE["nc"] = _build()
        if GATHER == "remote":
            # zero the semaphore file (previous NEFFs leave junk), then run
            # one discarded warmup execution: the very first execution after
            # NEFF load delivers remote SBUF writes unreliably (sem
            # increments arrive, some data lanes don't), so its output is
            # garbage; every later execution is sound.
            cl = _build_cleaner()
            run_bass_kernel_spmd(cl, [{} for _ in range(NCORES)],
                                 core_ids=list(range(NCORES)))
            run_bass_kernel_spmd(_CACHE["nc"], in_maps,
                                 core_ids=list(range(NCORES)))
    nc = _CACHE["nc"]
    res = run_bass_kernel_spmd(nc, in_maps, core_ids=list(range(NCORES)))
    parts = [res.results[r]["out"].transpose(0, 2, 1)       # [16, 128, 2]
             for r in range(NCORES)]
    full = np.concatenate(parts, axis=1)                    # [16, 1024, 2]
    return np.ascontiguousarray(full.reshape(B, 2048, 1).astype(np.float32))


# revision 48
# speedup vs baseline: 1.2231x; 1.0807x over previous
"""Trainium2 Bass kernel for the butterfly-CNN problem (nn_CNNLayer_30296699306356).

Network (see problem reference): input conv (k=2,s=2, 1->8 ch) + 10 butterfly
conv levels (k=2,s=2, channels double each level, relu, zero biases) + a
per-block dense matmul (1024 blocks of [8,2]) at the end.

Strategy (memory-regime; weights are ~358 MB fp32 dominated by levels 8-10):
  - Levels 5..9 run in bf16, level 10 in fp8 e3m4 (scale 4, folded into
    fea_dense); activations bf16 from level 5 on, fp32 PSUM accumulation.
    Measured rel err vs the fp32 reference: ~1.4e-2 (threshold 2e-2).
  - Levels in..8 are replicated on all 8 cores; levels 9/10 shard the OUTPUT
    channels across the cores (1/8 of the dominant weight traffic per core).
  - The x9 shard exchange (each core's 512 channels -> all cores) is done
    with direct peer-to-peer SWDGE remote DMAs (XOR slot layout: slot d on
    core r holds the shard of core r^d), NOT an NRT collective: the NRT
    AllGather costs ~40 us of latency; the raw remote DMA exchange costs a
    few us. A compile-time-inserted prelude AllGather (bir_kernel_barrier)
    makes the cross-core SBUF writes safe. Per-slot semaphores let level 10
    consume each peer shard as it lands (slot-pipelined contraction).
  - Level 10's weight chunks are pre-permuted per core into slot order, so
    the contraction walks slots 0..7 (own shard first, no wait).
  - DMA queue split: the big weight stream (wmid/w9/w10) runs on the sync
    HWDGE queue with enough pool buffers that it never stalls; w8 (whose
    pool WAR-waits on L8 compute) runs on the scalar HWDGE queue so it
    cannot head-of-line-block the stream.
  - Final block einsum on the vector engine (broadcast mult + grouped
    reduce); output layout [B, o, block], transposed on host.

kernel(**inputs) takes the FULL unsharded inputs and returns the FULL output.
"""

import ml_dtypes
import numpy as np

NCORES = 8
B = 16
P = 128
C = 8
NLVL = 10
BF16 = ml_dtypes.bfloat16
E3M4 = ml_dtypes.float8_e3m4
W10_SCALE = 4.0  # f10 quantized as e3m4(4*w); 1/4 folded into fea_dense
GATHER = "cc"  # "remote" (peer DMA exchange) or "cc" (NRT AllGather)
# slot d on core r holds the x9 shard of core PERM[r][d] = r ^ PSI[d]; PSI
# measured empirically with a rank-broadcast probe kernel (the driver's
# logical->physical nc remap makes the relative-XOR addressing land slots
# 4..7 with bit 1 flipped: cross-die first hops swap the RMTV lane pair)
PSI = [0, 1, 2, 3, 6, 7, 4, 5]
PERM = [[r ^ PSI[d] for d in range(NCORES)] for r in range(NCORES)]

_CACHE = {}


# ---------------------------------------------------------------- host prep

def _host_prep(inputs):
    """Build the per-core input maps (numpy only)."""
    ind = np.ascontiguousarray(np.asarray(inputs["in_data"], np.float32))
    f = {l: np.asarray(inputs[f"f{l}"], np.float32) for l in range(1, NLVL + 1)}
    f0 = np.asarray(inputs["in_filter"], np.float32)     # [2, 1, 8]
    fd = np.asarray(inputs["fea_dense"], np.float32)     # [1024, 8, 2]

    shared = {}
    # r0 [32, 64, 16]: r0[row, wHi, b] = in[b, wHi*32 + row]
    shared["r0"] = np.ascontiguousarray(
        ind[:, :, 0].reshape(B, 64, 32).transpose(2, 1, 0))

    # w0 [32, 128]: rows (2*wsub + k), cols (wsub*8 + co)
    w0 = np.zeros((32, 128), np.float32)
    for wsub in range(16):
        for k in range(2):
            w0[2 * wsub + k, wsub * 8:wsub * 8 + 8] = f0[k, 0, :]
    shared["w0"] = w0

    # packed levels 1..4 stacked: wpk [4, 128, 128]
    wpk = np.zeros((4, 128, 128), np.float32)
    for lvl in range(1, 5):
        cin = 2 ** (lvl - 1) * C
        cout = 2 ** lvl * C
        s_out = (128 // cin) // 2
        for wso in range(s_out):
            for k in range(2):
                wsi = 2 * wso + k
                wpk[lvl - 1, wsi * cin:(wsi + 1) * cin,
                    wso * cout:(wso + 1) * cout] = f[lvl][k]
    shared["wpk"] = wpk

    # w5/w6/w7 mega-packed [128, 10752] bf16 (kt-major per level), one DMA
    w5v = f[5].astype(BF16).reshape(2, 1, 128, 256)
    w6v = f[6].astype(BF16).reshape(2, 2, 128, 512)
    w7v = f[7].astype(BF16).reshape(2, 4, 128, 1024)
    shared["wmid"] = np.ascontiguousarray(np.concatenate([
        w5v.transpose(2, 0, 1, 3).reshape(128, 512),
        w6v.transpose(2, 0, 1, 3).reshape(128, 2048),
        w7v.transpose(2, 0, 1, 3).reshape(128, 8192)], axis=1))

    # f8/f9/f10 output-channel shards, packed into 4-ci-tile chunks:
    # [nchunks, 128, 4, cout_shard]; chunk m = k*(cin//512) + q, cit = q*4+j
    def shard_pack(fl, cin, dt=BF16):
        sh = fl.shape[-1] // NCORES
        out = []
        flb = fl.astype(dt)
        for r in range(NCORES):
            blk = flb[:, :, r * sh:(r + 1) * sh]
            v = blk.reshape(2, cin // 512, 4, 128, sh).transpose(0, 1, 3, 2, 4)
            out.append(np.ascontiguousarray(
                v.reshape(2 * (cin // 512), 128, 4, sh)))
        return out

    # f8 is REPLICATED (cheaper than a second exchange): co-major chunks
    # [4, 128, kt=16, co=512], kt = k*8 + cit
    f8b = f[8].astype(BF16)
    w8full = np.stack([
        np.ascontiguousarray(
            f8b[:, :, c * 512:(c + 1) * 512]
            .reshape(2, 8, 128, 512).transpose(2, 0, 1, 3).reshape(128, 16, 512))
        for c in range(4)])
    shared["w8"] = w8full

    w9s = shard_pack(f[9], 2048)                      # 8 chunks of [128, 4, 512]
    w10s = shard_pack(f[10] * W10_SCALE, 4096, E3M4)  # 16 chunks of [128, 4, 1024]
    # reorder per core into slot order: new[2d+k] = old[k*8 + slot_rank(d)];
    # in "cc" mode slot d of the gathered x9sb simply holds rank d
    perm = PERM if GATHER == "remote" else [list(range(NCORES))] * NCORES
    w10s = [np.ascontiguousarray(
                w[[k * 8 + perm[r][d] for d in range(8) for k in range(2)]])
            for r, w in enumerate(w10s)]

    # fea_dense shard, per-o flattened, tiled over the 16 batch partitions;
    # carries the 1/W10_SCALE compensation for the e3m4 level-10 weights
    fds = []
    for r in range(NCORES):
        blk = fd[r * 128:(r + 1) * 128] / W10_SCALE        # [128, 8, 2]
        flat = blk.transpose(2, 0, 1).reshape(2, 1024)     # [o, 1024]
        fds.append(np.ascontiguousarray(
            np.broadcast_to(flat[None], (B, 2, 1024))))

    in_maps = []
    for r in range(NCORES):
        m = dict(shared)
        m["w9"] = w9s[r]
        m["w10"] = w10s[r]
        m["fdt"] = fds[r]
        in_maps.append(m)
    return in_maps


# ---------------------------------------------------------------- bass build

def _build():
    import concourse.bass as bass
    import concourse.mybir as mybir
    import concourse.tile as tile
    from concourse import bacc, library_config

    f32 = mybir.dt.float32
    bf16 = mybir.dt.bfloat16
    fp8 = mybir.dt.float8e3
    RELU = mybir.ActivationFunctionType.Relu

    nc = bacc.Bacc("TRN2", target_bir_lowering=False, debug=False,
                   num_devices=NCORES)

    def inp(name, shape, dt=f32):
        return nc.dram_tensor(name, shape, dt, kind="ExternalInput").ap()

    r0 = inp("r0", [32, 64, 16])
    w0 = inp("w0", [32, 128])
    wpk = inp("wpk", [4, 128, 128])
    wmid = inp("wmid", [128, 10752], bf16)
    w8 = inp("w8", [4, 128, 16, 512], bf16)
    w9 = inp("w9", [8, 128, 4, 512], bf16)
    w10 = inp("w10", [16, 128, 4, 1024], fp8)
    fdt = inp("fdt", [B, 2, 1024])
    out = nc.dram_tensor("out", [B, 2, 128], f32, kind="ExternalOutput").ap()

    remote = GATHER == "remote"

    with tile.TileContext(nc) as tc:
        with (
            tc.tile_pool(name="const", bufs=1) as constp,
            tc.tile_pool(name="actp", bufs=3) as actp,
            tc.tile_pool(name="bigp", bufs=1) as bigp,
            tc.tile_pool(name="w7p", bufs=1) as w7p,
            tc.tile_pool(name="w8p", bufs=2) as w8p,
            tc.tile_pool(name="w9p", bufs=6) as w9p,
            tc.tile_pool(name="w10p", bufs=16) as w10p,
            tc.tile_pool(name="psA", bufs=2, space="PSUM") as psA,
            tc.tile_pool(name="psB", bufs=4, space="PSUM") as psB,
            tc.tile_pool(name="psC", bufs=2, space="PSUM") as psC,
            tc.tile_pool(name="dramp", bufs=1, space="DRAM") as dramp,
        ):
            if remote:
                rsems = [nc.alloc_semaphore(f"xch{d}") for d in range(1, NCORES)]
                lsem = nc.alloc_semaphore("xch_local")
                x9rdy = nc.alloc_semaphore("x9rdy")
                nc.gpsimd.load_library(library_config.remote_dma)
                slot_mms = {d: [] for d in range(1, NCORES)}

            if remote:
                # exchange buffers allocated up front so the SWDGE descriptor
                # generation (slow: ~1.7us per broadcast prep on Q7) can run
                # during the initial weight streaming; the data dependency on
                # x9loc defers to the trigger
                x9loc = bigp.tile([128, 4, 2, 16], bf16, name="x9loc")
                x9sb = bigp.tile([128, 32, 2, 16], bf16, name="x9sb")
                preps = []
                for d in range(1, NCORES):
                    rdests = [None] * NCORES
                    rdests[d] = (0, d)
                    preps.append(nc.gpsimd.remote_dma_broadcast(
                        x9sb[:, 4 * d:4 * d + 4], x9loc[:],
                        remote_sem=rsems[d - 1], local_sem=lsem,
                        rdests=rdests))

            # ---- resident loads (sync queue = the main weight stream)
            r0sb = constp.tile([32, 64, 16], f32, name="r0sb")
            nc.sync.dma_start(r0sb[:], r0)
            w0sb = constp.tile([32, 128], f32, name="w0sb")
            nc.sync.dma_start(w0sb[:], w0)
            wpksb = constp.tile([128, 4, 128], f32, name="wpksb")
            nc.sync.dma_start(wpksb[:], wpk.rearrange("l p c -> p l c"))
            wmidsb = w7p.tile([128, 10752], bf16, name="wmidsb")
            nc.sync.dma_start(wmidsb[:], wmid)
            w5sb = wmidsb[:, 0:512].rearrange("p (t c) -> p t c", c=256)
            w6sb = wmidsb[:, 512:2560].rearrange("p (t c) -> p t c", c=512)
            w7sb = wmidsb[:, 2560:10752].rearrange("p (t c) -> p t c", c=1024)
            fdsb = constp.tile([B, 2, 1024], f32, name="fdsb")
            nc.scalar.dma_start(fdsb[:], fdt)

            # ---- input conv + packed levels 1..4 (all [128, 64, 16])
            xprev = None
            for lvl in range(5):
                # x4 feeds the bf16 level-5 matmul, so cast at the relu
                xn = actp.tile([128, 64, 16], bf16 if lvl == 4 else f32,
                               name=f"x{lvl}", tag="xl")
                for ch in range(2):
                    ps = psA.tile([128, 32, 16], f32, name="psA", tag="psA")
                    if lvl == 0:
                        nc.tensor.matmul(
                            ps[:], w0sb[:], r0sb[:, ch * 32:(ch + 1) * 32, :],
                            start=True, stop=True)
                    else:
                        nc.tensor.matmul(
                            ps[:], wpksb[:, lvl - 1, :],
                            xprev[:, ch * 32:(ch + 1) * 32, :],
                            start=True, stop=True)
                    nc.scalar.activation(
                        xn[:, ch * 32:(ch + 1) * 32, :], ps[:], RELU)
                xprev = xn

            # ---- standard levels (orientation A, weights stationary)
            def std_level(xin, wsb, cin_t, cout_t, w_out, name, out_tile=None):
                # xin [128, cin_t, 2*w_out, 16]; wsb [128, 2*cin_t, co] with
                # kt = k*cin_t + cit; returns [128, cout_t, w_out, 16]
                if out_tile is None:
                    xn = actp.tile([128, cout_t, w_out, 16], bf16,
                                   name=name, tag="xl")
                else:
                    xn = out_tile
                for ct in range(cout_t):
                    ps = psA.tile([128, w_out, 16], f32, name="psA", tag="psA")
                    for cit in range(cin_t):
                        rhs2 = xin[:, cit].rearrange(
                            "p (w two) b -> p two w b", two=2)
                        for k in range(2):
                            nc.tensor.matmul(
                                ps[:],
                                wsb[:, k * cin_t + cit,
                                    ct * 128:(ct + 1) * 128],
                                rhs2[:, k],
                                start=(cit == 0 and k == 0),
                                stop=(cit == cin_t - 1 and k == 1))
                    nc.scalar.activation(xn[:, ct], ps[:], RELU)
                return xn

            x5 = std_level(xprev[:, None], w5sb, 1, 2, 32, "x5")
            x6 = std_level(x5, w6sb, 2, 4, 16, "x6")
            x7 = std_level(x6, w7sb, 4, 8, 8, "x7")

            # ---- level 8 REPLICATED (full 2048 cout), co-major weight stream
            # w8 chunks stream on the scalar queue: their pool WAR-waits on L8
            # compute and must not block the sync queue's w9/w10 stream.
            x8sb = bigp.tile([128, 16, 4, 16], bf16, name="x8sb")
            for c in range(4):
                w8c = w8p.tile([128, 16, 512], bf16, name="w8c", tag="w8c")
                nc.sync.dma_start(w8c[:], w8[c])
                for ctl in range(4):
                    ps = psA.tile([128, 4, 16], f32, name="psA", tag="psA")
                    for cit in range(8):
                        rhs2 = x7[:, cit].rearrange(
                            "p (w two) b -> p two w b", two=2)
                        for k in range(2):
                            nc.tensor.matmul(
                                ps[:],
                                w8c[:, k * 8 + cit, ctl * 128:(ctl + 1) * 128],
                                rhs2[:, k],
                                start=(cit == 0 and k == 0),
                                stop=(cit == 7 and k == 1))
                    nc.scalar.activation(x8sb[:, c * 4 + ctl], ps[:], RELU)

            # ---- level 9 (512-ch shard, streamed weights, 4 accumulators)
            ps9 = [psB.tile([128, 2, 16], f32, name=f"ps9_{ct}", tag="psB")
                   for ct in range(4)]
            for m in range(8):
                w9c = w9p.tile([128, 4, 512], bf16, name="w9c", tag="w9c")
                nc.sync.dma_start(w9c[:], w9[m])
                k, q = divmod(m, 4)
                for j in range(4):
                    cit = q * 4 + j
                    rhs = x8sb[:, cit].rearrange(
                        "p (w two) b -> p two w b", two=2)[:, k]
                    for ct in range(4):
                        nc.tensor.matmul(
                            ps9[ct][:],
                            w9c[:, j, ct * 128:(ct + 1) * 128],
                            rhs,
                            start=(m == 0 and j == 0),
                            stop=(m == 7 and j == 3))
            if not remote:
                x9loc = bigp.tile([128, 4, 2, 16], bf16, name="x9loc")
            for ct in range(4):
                nc.scalar.activation(x9loc[:, ct], ps9[ct][:], RELU)
            if remote:
                # The early-emitted SWDGE preps had no x9loc producer to
                # defer a RAW edge from, so trigger1 would fire before x9loc
                # exists. A reader spanning all four relu outputs bumps an
                # explicit readiness semaphore for it instead (the relus'
                # own sync-update slots are taken).
                x9scr = bigp.tile([128, 4, 16], bf16, name="x9scr")
                x9cp = nc.vector.tensor_copy(x9scr[:], x9loc[:, :, 1, :])

            # ---- x9 shard exchange: push my shard into slot d of peer
            # me^d; peer's slot d gets +2 on rsems[d-1] when the bytes have
            # landed. A second redundant send round (same bytes, same sems)
            # covers the race where a late-starting peer's start-of-kernel
            # sem clear wipes the first round's increments.
            if remote:
                trig1 = nc.gpsimd.trigger_dma(count=None)
                for d in range(1, NCORES):
                    rdests = [None] * NCORES
                    rdests[d] = (0, d)
                    nc.gpsimd.remote_dma_broadcast(
                        x9sb[:, 4 * d:4 * d + 4], x9loc[:],
                        remote_sem=rsems[d - 1], local_sem=lsem,
                        rdests=rdests)
                nc.gpsimd.trigger_dma(count=None)
            else:
                # two pipelined half-gathers: half A (k-tiles 0,1 of every
                # rank's shard) fires after only two of the four L9 relus,
                # and L10's phase-1 matmuls run under half B's latency
                x9sb = bigp.tile([128, 32, 2, 16], bf16, name="x9sb")
                x9v = x9sb.rearrange("p (r t) two b -> p r t two b", r=NCORES)
                for h in range(2):
                    agi = dramp.tile([128, 2, 2, 16], bf16, name=f"ag9i{h}")
                    ago = dramp.tile([NCORES, 128, 2, 2, 16], bf16,
                                     name=f"ag9o{h}", addr_space="Shared")
                    nc.scalar.dma_start(agi[:], x9loc[:, 2 * h:2 * h + 2])
                    nc.gpsimd.collective_compute(
                        "AllGather", mybir.AluOpType.bypass,
                        replica_groups=[list(range(NCORES))],
                        ins=[agi.opt()], outs=[ago.opt()])
                    nc.scalar.dma_start(
                        x9v[:, :, 2 * h:2 * h + 2],
                        ago.rearrange("r p t two b -> p r t two b"))

            # ---- level 10 (1024-ch shard, orientation B: acts stationary,
            #      e3m4 weights moving, slot-pipelined over peer shards)
            ps10 = [psC.tile([B, 512], f32, name=f"ps10_{cb}", tag="psC")
                    for cb in range(2)]
            chunks = []
            for m in range(16):
                w10c = w10p.tile([128, 4, 1024], fp8, name="w10c",
                                 tag="w10c")
                nc.sync.dma_start(w10c[:], w10[m])
                chunks.append(w10c)
            if remote:
                for d in range(NCORES):
                    src, base = (x9loc, 0) if d == 0 else (x9sb, 4 * d)
                    for k in range(2):
                        w10c = chunks[2 * d + k]
                        for j in range(4):
                            lhsT = src[:, base + j, k, :]
                            for cb in range(2):
                                mm = nc.tensor.matmul(
                                    ps10[cb][:], lhsT,
                                    w10c[:, j, cb * 512:(cb + 1) * 512],
                                    start=(d == 0 and k == 0 and j == 0),
                                    stop=(d == NCORES - 1 and k == 1
                                          and j == 3))
                                if d > 0 and k == 0 and j == 0:
                                    slot_mms[d].append(mm)
            else:
                # phase 0 consumes half-gather A (j-tiles 0,1), phase 1
                # consumes half B (j-tiles 2,3) -- phase 0 runs under half
                # B's collective latency
                for ph in range(2):
                    for d in range(NCORES):
                        for k in range(2):
                            w10c = chunks[2 * d + k]
                            for j in (2 * ph, 2 * ph + 1):
                                lhsT = x9sb[:, 4 * d + j, k, :]
                                for cb in range(2):
                                    nc.tensor.matmul(
                                        ps10[cb][:], lhsT,
                                        w10c[:, j, cb * 512:(cb + 1) * 512],
                                        start=(ph == 0 and d == 0 and k == 0
                                               and j == 0),
                                        stop=(ph == 1 and d == NCORES - 1
                                              and k == 1 and j == 3))
            x10 = bigp.tile([B, 1024], f32, name="x10")
            for cb in range(2):
                nc.scalar.activation(
                    x10[:, cb * 512:(cb + 1) * 512], ps10[cb][:], RELU)

            # ---- final per-block einsum on the vector engine
            # osb laid out [B, o, k]; host transposes to [B, k, o]
            prod = bigp.tile([B, 2, 1024], f32, name="prod")
            osb = bigp.tile([B, 2, 128], f32, name="osb")
            x10b = x10.rearrange("p (one f) -> p one f", one=1).broadcast_to(
                [B, 2, 1024])
            nc.vector.tensor_tensor(
                prod[:], x10b, fdsb[:], mybir.AluOpType.mult)
            nc.vector.tensor_reduce(
                osb[:],
                prod.rearrange("p o (k c) -> p o k c", c=8),
                mybir.AxisListType.X, mybir.AluOpType.add)
            nc.scalar.dma_start(out, osb[:])

    if remote:
        # Cross-core waits are attached AFTER the tile block: the tile
        # scheduling pass simulates a single core and would report the
        # remote-satisfied semaphores as a deadlock.
        #  - a PE-stream wait right before the first matmul touching slot d
        #    gates the whole slot on the peer shard having landed (+2 on
        #    rsems[d-1]); inserted directly into the lowered instruction
        #    list (the matmuls' own wait slots are already taken by the
        #    tile-assigned weight-chunk arrival waits)
        def find_block(ins):
            for blk in nc.main_func.blocks:
                for i, x in enumerate(blk.instructions):
                    if x.name == ins.name:
                        return blk, i
            raise KeyError(ins.name)

        for d in range(1, NCORES):
            w = nc.tensor.wait_ge(rsems[d - 1], 2).ins
            wblk, widx = find_block(w)
            del wblk.instructions[widx]
            # insert before whichever of the slot's chain-first matmuls the
            # scheduler placed earliest
            places = [find_block(mm.ins) for mm in slot_mms[d]]
            mblk, midx = min(places, key=lambda bi: bi[1])
            assert all(b is mblk for b, _ in places)
            mblk.instructions.insert(midx, w)
        #  - trigger1 must not fire descriptors before x9loc is computed.
        #    The x9scr copy reads all four relu outputs; a nop-with-update
        #    placed right after it on the in-order vector stream bumps x9rdy
        #    (no instruction has a free sync-update slot for a second
        #    update, so the increment needs its own instruction)
        ni = nc.vector.nop(nofuse=True).then_inc(x9rdy, 1)
        niblk, niidx = find_block(ni.ins)
        del niblk.instructions[niidx]
        cblk, cidx = find_block(x9cp.ins)
        cblk.instructions.insert(cidx + 1, ni.ins)
        wt = nc.gpsimd.wait_ge(x9rdy, 1).ins
        wblk, widx = find_block(wt)
        del wblk.instructions[widx]
        tblk, tidx = find_block(trig1.ins)
        tblk.instructions.insert(tidx, wt)
        # Register the kernel-entry barrier replica group so compile()
        # inserts its prelude AllGather. NOTHING in the program waits on it:
        # it exists purely so the NEFF contains a collective, which makes
        # the runtime rendezvous the 8 cores' execution starts (without it,
        # profiler arming staggers the starts by milliseconds and every
        # early core idle-waits that long for peer shards).
        nc._bir_kernel_barrier_sem_replica_groups.append(set(range(NCORES)))
        # End-of-kernel drains: hold the epilogue semaphore zeroing until
        # every in-flight increment has landed -- both rounds of every peer's
        # push to me (sem_d reaches 4) and my own sends' local ticks. This
        # makes the between-executions state clean for any launch skew: once
        # a semaphore reads its final value nothing else is in flight, so
        # the epilogue clear cannot lose increments.
        for d in range(1, NCORES):
            nc.gpsimd.wait_ge(rsems[d - 1], 4)
        nc.gpsimd.wait_ge(lsem, 2 * 16 * (NCORES - 1))

    nc.compile()
    return nc


def _build_cleaner():
    """A trivial one-shot NEFF that zeroes the kernel semaphore range.

    The main kernel's cross-core exchange assumes its semaphores start at 0.
    Executions N>=1 get that from execution N-1's epilogue clear (held back
    by the end-of-kernel drains until all increments landed), but the very
    first execution after NEFF load would see whatever junk previous NEFFs
    left in the semaphore file. Running this cleaner once before the first
    real execution makes execution 0 sound too.
    """
    import concourse.mybir as mybir
    import concourse.tile as tile
    from concourse import bacc

    f32 = mybir.dt.float32
    nc = bacc.Bacc("TRN2", target_bir_lowering=False, debug=False,
                   num_devices=NCORES)
    out = nc.dram_tensor("out", [1, 128], f32, kind="ExternalOutput")
    with tile.TileContext(nc) as tc:
        with tc.tile_pool(name="p", bufs=1) as pool:
            z = pool.tile([1, 128], f32, name="z")
            nc.vector.memset(z[:], 0.0)
            nc.sync.dma_start(out.ap(), z[:])
    c = nc.gpsimd.sem_clear(range(100, 255)).ins
    for blk in nc.main_func.blocks:
        for i, x in enumerate(blk.instructions):
            if x.name == c.name:
                del blk.instructions[i]
                break
    tgt = next(b for b in nc.main_func.blocks
               if "tile_context" in b.name and "end" not in b.name)
    tgt.instructions.insert(0, c)
    nc.compile()
    return nc


# ------------------------------------------------------------------- kernel

def kernel(**inputs):
    from concourse.bass_utils import run_bass_kernel_spmd

    in_maps = _host_prep(inputs)
    if "nc" not in _CACHE:
        _CACHE["nc"] = _build()
        if GATHER == "remote":
            # zero the semaphore file (previous NEFFs leave junk), then run
            # one discarded warmup execution: the very first execution after
            # NEFF load delivers remote SBUF writes unreliably (sem
            # increments arrive, some data lanes don't), so its output is
            # garbage; every later execution is sound.
            cl = _build_cleaner()
            run_bass_kernel_spmd(cl, [{} for _ in range(NCORES)],
                                 core_ids=list(range(NCORES)))
            run_bass_kernel_spmd(_CACHE["nc"], in_maps,
                                 core_ids=list(range(NCORES)))
    nc = _CACHE["nc"]
    res = run_bass_kernel_spmd(nc, in_maps, core_ids=list(range(NCORES)))
    parts = [res.results[r]["out"].transpose(0, 2, 1)       # [16, 128, 2]
             for r in range(NCORES)]
    full = np.concatenate(parts, axis=1)                    # [16, 1024, 2]
    return np.ascontiguousarray(full.reshape(B, 2048, 1).astype(np.float32))
